# revision 23
# baseline (speedup 1.0000x reference)
"""Trainium2 Bass kernel for nn_CrossAttentionFusion (cross-attention + BitLinear FFN).

Sharding: 8 cores = 4 batches x 2 sequence-halves (data parallel, no collectives).
Each core owns 1024 query tokens; computes K/V for its batch's full 2048 tokens.

v2: fp8 DoubleRow matmuls everywhere (QKV/O/PV/FFN), host-side ternarization of
W1/W2 (shipped as fp8 +-1/0), softmax denominator computed on the PE via a
ones-matmul, act_quant realized as a direct fp8 cast.
"""
import math
import numpy as np
from contextlib import ExitStack

import concourse.bass as bass
import concourse.bass_isa as bass_isa
import concourse.tile as tile
from concourse import bacc, mybir
from concourse.bass_utils import run_bass_kernel_spmd

F32 = mybir.dt.float32
F32R = mybir.dt.float32r
BF16 = mybir.dt.bfloat16
FP8 = mybir.dt.float8e4
AF = mybir.ActivationFunctionType
ALU = mybir.AluOpType
DR = mybir.MatmulPerfMode.DoubleRow

B, S, DS, DP, H = 4, 2048, 1024, 512, 8
DF = 4 * DS
HD = DS // H          # 128
TOK = 1024            # query tokens per core
N_CORES = 8
EPS = 1e-6
QK_SCALE = 1.0 / math.sqrt(HD)
HALF_PI = math.pi / 2.0
WS = 32.0             # fp8 scale for Wq/Wk/Wv/Wo
CTX_S = 64.0          # fp8 scale for ctx (folded into 1/den via ones=1/64)

P = 128
M_SEM = DS // P       # 8
M_PRO = DP // P       # 4
M_FF = DF // P        # 32
NT_K = S // P         # 16


def build_nc(debug_outs=False):
    nc = bacc.Bacc("TRN2", target_bir_lowering=False, debug=False, num_devices=N_CORES)

    semT = nc.dram_tensor("semT", [DS, TOK], F32, kind="ExternalInput").ap()
    proT = nc.dram_tensor("proT", [DP, S], BF16, kind="ExternalInput").ap()
    wq = nc.dram_tensor("wq", [P, M_SEM, M_SEM, P], FP8, kind="ExternalInput").ap()
    wk = nc.dram_tensor("wk", [P, M_PRO, M_SEM, P], FP8, kind="ExternalInput").ap()
    wv = nc.dram_tensor("wv", [P, M_PRO, DS], FP8, kind="ExternalInput").ap()
    wo = nc.dram_tensor("wo", [P, M_SEM, M_SEM, P], FP8, kind="ExternalInput").ap()
    w1q = nc.dram_tensor("w1q", [P, M_SEM, M_FF, P], FP8, kind="ExternalInput").ap()
    w2q = nc.dram_tensor("w2q", [P, M_FF, M_SEM, P], FP8, kind="ExternalInput").ap()
    gsem = nc.dram_tensor("gsem", [P, M_SEM], F32, kind="ExternalInput").ap()
    gpro = nc.dram_tensor("gpro", [P, M_PRO], F32, kind="ExternalInput").ap()
    gff = nc.dram_tensor("gff", [P, M_SEM], F32, kind="ExternalInput").ap()
    bq = nc.dram_tensor("bq", [P, M_SEM], F32, kind="ExternalInput").ap()
    bk = nc.dram_tensor("bk", [P, M_SEM], F32, kind="ExternalInput").ap()
    obias = nc.dram_tensor("obias", [P, M_SEM], F32, kind="ExternalInput").ap()
    ybias = nc.dram_tensor("ybias", [P, M_SEM], F32, kind="ExternalInput").ap()
    acos = nc.dram_tensor("acos", [P, M_FF], F32, kind="ExternalInput").ap()
    nrb2 = nc.dram_tensor("nrb2", [P, M_FF], F32, kind="ExternalInput").ap()
    dqs = nc.dram_tensor("dqs", [P, 1], F32, kind="ExternalInput").ap()
    outT = nc.dram_tensor("outT", [DS, TOK], F32, kind="ExternalOutput").ap()

    semT_r = semT.rearrange("(m p) t -> p m t", p=P)
    proT_r = proT.rearrange("(m p) t -> p m t", p=P)
    outT_r = outT.rearrange("(m p) t -> p m t", p=P)

    with tile.TileContext(nc) as tc, ExitStack() as top:
        persist = top.enter_context(tc.tile_pool(name="persist", bufs=1))

        ones = persist.tile([P, 1], BF16)
        nc.vector.memset(ones[:], 1.0)
        ones_row = persist.tile([1, P], BF16)
        nc.vector.memset(ones_row[:], 1.0)
        ones2_f8 = persist.tile([P, 2, 16], FP8)
        nc.vector.memset(ones2_f8[:], 1.0 / CTX_S)
        eps_t = persist.tile([1, 1], F32)
        nc.vector.memset(eps_t[:], EPS)
        hpi_t = persist.tile([P, 1], F32)
        nc.vector.memset(hpi_t[:], HALF_PI)

        gsem_sb = persist.tile([P, M_SEM], F32)
        gpro_sb = persist.tile([P, M_PRO], F32)
        gff_sb = persist.tile([P, M_SEM], F32)
        bq_sb = persist.tile([P, M_SEM], F32)
        bk_sb = persist.tile([P, M_SEM], F32)
        obias_sb = persist.tile([P, M_SEM], F32)
        ybias_sb = persist.tile([P, M_SEM], F32)
        acos_sb = persist.tile([P, M_FF], F32)
        nrb2_sb = persist.tile([P, M_FF], F32)
        dqs_sb = persist.tile([P, 1], F32)
        for ap_d, t in [(gsem, gsem_sb), (gpro, gpro_sb), (gff, gff_sb),
                        (bq, bq_sb), (bk, bk_sb), (obias, obias_sb),
                        (ybias, ybias_sb), (acos, acos_sb), (nrb2, nrb2_sb),
                        (dqs, dqs_sb)]:
            nc.sync.dma_start(t[:], ap_d[:])

        # persistent big tensors
        semT_sb = persist.tile([P, M_SEM, TOK], F32)      # becomes semout in place
        es_att = ExitStack()
        patt_io = es_att.enter_context(tc.tile_pool(name="patt_io", bufs=1))
        q_sb = patt_io.tile([P, M_SEM, TOK], FP8)
        k_sb = patt_io.tile([P, M_SEM, S], FP8)
        v_sb = patt_io.tile([P, NT_K, DS], FP8)
        ctx_n = [persist.tile([P, M_SEM, 512], FP8, name=f"ctx{n}")
                 for n in range(2)]

        for m2 in range(4):
            nc.sync.dma_start(semT_sb[:, 2 * m2:2 * m2 + 2, :],
                              semT_r[:, 2 * m2:2 * m2 + 2, :])

        # ---------- input norms -> semn/pron in fp8 ----------
        es_norm = ExitStack()
        pnorm = es_norm.enter_context(tc.tile_pool(name="pnorm", bufs=1))
        semn = pnorm.tile([P, M_SEM, TOK], FP8)
        pron = pnorm.tile([P, M_PRO, S], FP8)
        ps_nrm = es_norm.enter_context(
            tc.tile_pool(name="ps_nrm", bufs=2, space="PSUM"))

        with tc.tile_pool(name="pnsc", bufs=1) as pnsc:
            proT_sb = pnsc.tile([P, M_PRO, S], BF16)
            for m in range(M_PRO):
                nc.sync.dma_start(proT_sb[:, m, :], proT_r[:, m, :])

            def rmsnorm(x_sb, nm, T, g, out_f8):
                D = nm * P
                sq = pnsc.tile([P, nm * T], BF16, tag="sq", bufs=1, name="sq")
                xf = x_sb[:].rearrange("p a b -> p (a b)")
                half = nm * T // 2
                nc.scalar.activation(sq[:, :half], xf[:, :half], AF.Square)
                nc.scalar.activation(sq[:, half:], xf[:, half:], AF.Square)
                rs_row = pnsc.tile([1, T], F32, tag=f"rs{nm}", bufs=1)
                for ch in range(T // 512):
                    pst = ps_nrm.tile([P, 512], F32, tag="nrm")
                    ps = pst[0:1, :]
                    for m in range(nm):
                        nc.tensor.matmul(
                            ps[:], ones[:],
                            sq[:, m * T + ch * 512:m * T + (ch + 1) * 512],
                            start=(m == 0), stop=(m == nm - 1))
                    nc.scalar.activation(rs_row[:, ch * 512:(ch + 1) * 512],
                                         ps[:], AF.Ln, bias=eps_t[:],
                                         scale=1.0 / D)
                nc.scalar.activation(rs_row[:], rs_row[:], AF.Exp, scale=-0.5)
                rs_bc = pnsc.tile([P, T], F32, tag=f"rsbc{nm}", bufs=1)
                nc.gpsimd.partition_broadcast(rs_bc[:], rs_row[:])
                for m in range(nm):
                    nc.vector.scalar_tensor_tensor(
                        out=out_f8[:, m, :], in0=x_sb[:, m, :],
                        scalar=g[:, m:m + 1], in1=rs_bc[:],
                        op0=ALU.mult, op1=ALU.mult)

            rmsnorm(semT_sb, M_SEM, TOK, gsem_sb, semn)
            rmsnorm(proT_sb, M_PRO, S, gpro_sb, pron)

        # ---------- QKV projections (fp8 DoubleRow) ----------
        es_qw = ExitStack()
        pqw = es_qw.enter_context(tc.tile_pool(name="pqw", bufs=1, side="right"))
        wq_sb = pqw.tile([P, M_SEM, M_SEM, P], FP8)
        wk_sb = pqw.tile([P, M_PRO, M_SEM, P], FP8)
        wv_sb = pqw.tile([P, M_PRO, DS], FP8)
        nc.sync.dma_start(wq_sb[:], wq[:])
        nc.sync.dma_start(wk_sb[:], wk[:])
        nc.sync.dma_start(wv_sb[:], wv[:])

        with tc.tile_pool(name="ps_mm", bufs=2, space="PSUM") as ps_mm:
            # Q: out q_sb[:, m, 0:1024]
            for m in range(M_SEM):
                ps = ps_mm.tile([P, 1024], F32, tag="mm")
                for n2 in range(2):
                    for kk in range(4):
                        nc.tensor.matmul(
                            ps[:, n2 * 512:(n2 + 1) * 512],
                            wq_sb[:, 2 * kk:2 * kk + 2, m, :],
                            semn[:, 2 * kk:2 * kk + 2, n2 * 512:(n2 + 1) * 512],
                            start=(kk == 0), stop=(kk == 3), perf_mode=DR)
                nc.vector.tensor_scalar(q_sb[:, m, :], ps[:], 1.0 / WS,
                                        bq_sb[:, m:m + 1], ALU.mult, ALU.add)
            # K: out k_sb[:, m, 0:2048]
            for m in range(M_SEM):
                for c2 in range(2):
                    ps = ps_mm.tile([P, 1024], F32, tag="mm")
                    for n2 in range(2):
                        col = (c2 * 2 + n2) * 512
                        for kk in range(2):
                            nc.tensor.matmul(
                                ps[:, n2 * 512:(n2 + 1) * 512],
                                wk_sb[:, 2 * kk:2 * kk + 2, m, :],
                                pron[:, 2 * kk:2 * kk + 2, col:col + 512],
                                start=(kk == 0), stop=(kk == 1), perf_mode=DR)
                    nc.scalar.activation(k_sb[:, m, c2 * 1024:(c2 + 1) * 1024],
                                         ps[:], AF.Identity, scale=1.0 / WS,
                                         bias=bk_sb[:, m:m + 1])
            # V (transposed, kpos-major): out v_sb[:, mt, 0:1024]
            for mt in range(NT_K):
                ps = ps_mm.tile([P, 1024], F32, tag="mm")
                for n2 in range(2):
                    for kk in range(2):
                        nc.tensor.matmul(
                            ps[:, n2 * 512:(n2 + 1) * 512],
                            pron[:, 2 * kk:2 * kk + 2, mt * P:(mt + 1) * P],
                            wv_sb[:, 2 * kk:2 * kk + 2, n2 * 512:(n2 + 1) * 512],
                            start=(kk == 0), stop=(kk == 1), perf_mode=DR)
                nc.vector.tensor_scalar(v_sb[:, mt, :], ps[:], 1.0 / WS, None,
                                        ALU.mult)
        es_norm.close()
        es_qw.close()

        # FFN weights arrive during attention
        es_fw = ExitStack()
        pfw = es_fw.enter_context(tc.tile_pool(name="pfw", bufs=1, side="right"))
        w1q_sb = pfw.tile([P, M_SEM, M_FF, P], FP8)
        w2q_sb = pfw.tile([P, M_FF, M_SEM, P], FP8)
        for c4 in range(4):
            nc.sync.dma_start(w1q_sb[:, 2 * c4:2 * c4 + 2], w1q[:, 2 * c4:2 * c4 + 2])
        for c4 in range(4):
            nc.sync.dma_start(w2q_sb[:, 8 * c4:8 * c4 + 8], w2q[:, 8 * c4:8 * c4 + 8])
        wo_sb = pfw.tile([P, M_SEM, M_SEM, P], FP8)
        nc.sync.dma_start(wo_sb[:], wo[:])

        # ---------- attention ----------
        with tc.tile_pool(name="pattn", bufs=1) as pattn, \
             tc.tile_pool(name="ps_att", bufs=1, space="PSUM") as ps_att:
            for n in range(2):
                for h in range(H):
                    pt = pattn.tile([P, NT_K, 512], FP8, tag="pt", bufs=2)
                    den = ps_att.tile([P, 512], F32, tag="den", bufs=2)
                    cps = ps_att.tile([P, 512], F32, tag="pv", bufs=2)
                    for kp in range(8):
                        sc = ps_att.tile([P, 1024], F32, tag="sc", bufs=2)
                        nc.tensor.matmul(
                            sc[:, 0:512],
                            k_sb[:, h, (2 * kp) * P:(2 * kp + 1) * P],
                            q_sb[:, h, n * 512:(n + 1) * 512],
                            start=True, stop=True)
                        nc.tensor.matmul(
                            sc[:, 512:1024],
                            k_sb[:, h, (2 * kp + 1) * P:(2 * kp + 2) * P],
                            q_sb[:, h, n * 512:(n + 1) * 512],
                            start=True, stop=True)
                        nc.scalar.activation(
                            pt[:, 2 * kp:2 * kp + 2, :].rearrange(
                                "p a b -> p (a b)"),
                            sc[:], AF.Exp, scale=QK_SCALE)
                    for kp in range(8):
                        nc.tensor.matmul(den[0:1, :], ones2_f8[:, :, 0:1],
                                         pt[:, 2 * kp:2 * kp + 2, :],
                                         start=(kp == 0), stop=(kp == 7),
                                         perf_mode=DR)
                    for kp in range(8):
                        nc.tensor.matmul(cps[:],
                                         v_sb[:, 2 * kp:2 * kp + 2,
                                              h * P:(h + 1) * P],
                                         pt[:, 2 * kp:2 * kp + 2, :],
                                         start=(kp == 0), stop=(kp == 7),
                                         perf_mode=DR)
                    rden = pattn.tile([1, 512], F32, tag="rden", bufs=2)
                    nc.vector.reciprocal_approx_fast(rden[:], den[0:1, :])
                    bc_sb = pattn.tile([P, 512], F32, tag="bcsb", bufs=2)
                    nc.gpsimd.partition_broadcast(bc_sb[:], rden[:])
                    nc.vector.tensor_tensor(ctx_n[n][:, h, :], cps[:], bc_sb[:],
                                            op=ALU.mult)
        es_att.close()

        # ---------- out-proj + ffn-norm + FFN (per half) ----------
        with tc.tile_pool(name="pff", bufs=1) as pff, \
             tc.tile_pool(name="ps_ff", bufs=2, space="PSUM") as ps_ff:
            xq_n = [pff.tile([P, M_SEM, 512], FP8, name=f"xq{n}")
                    for n in range(2)]
            h2_n = [pff.tile([P, M_FF, 512], FP8, name=f"h2{n}")
                    for n in range(2)]
            for n in range(2):
                ncol = slice(n * 512, (n + 1) * 512)
                # out-proj
                t_o = pff.tile([P, M_SEM, 512], BF16, tag="t_o", bufs=1)
                for m in range(M_SEM):
                    ps = ps_ff.tile([P, 512], F32, tag="mmf")
                    for kk in range(4):
                        nc.tensor.matmul(ps[:], wo_sb[:, 2 * kk:2 * kk + 2, m, :],
                                         ctx_n[n][:, 2 * kk:2 * kk + 2, :],
                                         start=(kk == 0), stop=(kk == 3),
                                         perf_mode=DR)
                    nc.vector.tensor_scalar(t_o[:, m, :], ps[:],
                                            1.0 / (WS * CTX_S),
                                            obias_sb[:, m:m + 1],
                                            ALU.mult, ALU.add)
                semo = semT_sb[:, :, ncol]
                nc.vector.tensor_tensor(semo, t_o[:], semo, op=ALU.add)

                # ffn norm -> xq (fp8)
                sqf = pff.tile([P, M_SEM, 512], BF16, tag="sqf", bufs=1)
                nc.gpsimd.tensor_tensor(sqf[:], semo, semo, op=ALU.mult)
                pst = ps_ff.tile([P, 512], F32, tag="nrmf")
                psr = pst[0:1, :]
                for m in range(M_SEM):
                    nc.tensor.matmul(psr[:], ones[:], sqf[:, m, :],
                                     start=(m == 0), stop=(m == M_SEM - 1))
                rsf = pff.tile([1, 512], F32, tag="rsf", bufs=1)
                nc.scalar.activation(rsf[:], psr[:], AF.Ln, bias=eps_t[:],
                                     scale=1.0 / DS)
                nc.scalar.activation(rsf[:], rsf[:], AF.Exp, scale=-0.5)
                rsbc = pff.tile([P, 512], F32, tag="rsbc", bufs=1)
                nc.gpsimd.partition_broadcast(rsbc[:], rsf[:])
                for m in range(M_SEM):
                    nc.vector.scalar_tensor_tensor(
                        out=xq_n[n][:, m, :], in0=semT_sb[:, m, ncol],
                        scalar=gff_sb[:, m:m + 1], in1=rsbc[:],
                        op0=ALU.mult, op1=ALU.mult)

                # FFN1 + snake
                for m in range(M_FF):
                    ps = ps_ff.tile([P, 512], F32, tag="mmf")
                    for kk in range(4):
                        nc.tensor.matmul(ps[:], w1q_sb[:, 2 * kk:2 * kk + 2, m, :],
                                         xq_n[n][:, 2 * kk:2 * kk + 2, :],
                                         start=(kk == 0), stop=(kk == 3),
                                         perf_mode=DR)
                    sn = pff.tile([P, 512], BF16, tag="sn", bufs=3)
                    nc.scalar.activation(sn[:], ps[:], AF.Sin, bias=hpi_t[:],
                                         scale=acos_sb[:, m:m + 1])
                    nc.vector.scalar_tensor_tensor(
                        out=h2_n[n][:, m, :], in0=sn[:],
                        scalar=nrb2_sb[:, m:m + 1], in1=ps[:],
                        op0=ALU.mult, op1=ALU.add)

            # FFN2 (both halves; h2 ready per half in order)
            for n in range(2):
                ncol = slice(n * 512, (n + 1) * 512)
                yf = pff.tile([P, M_SEM, 512], BF16, tag="yf", bufs=1)
                for m in range(M_SEM):
                    ps = ps_ff.tile([P, 512], F32, tag="mmf")
                    for kk in range(16):
                        nc.tensor.matmul(ps[:], w2q_sb[:, 2 * kk:2 * kk + 2, m, :],
                                         h2_n[n][:, 2 * kk:2 * kk + 2, :],
                                         start=(kk == 0), stop=(kk == 15),
                                         perf_mode=DR)
                    nc.vector.tensor_scalar(yf[:, m, :], ps[:], dqs_sb[:],
                                            ybias_sb[:, m:m + 1],
                                            ALU.mult, ALU.add)
                yo = pff.tile([P, M_SEM, 512], F32, tag="yo", bufs=1)
                nc.vector.tensor_tensor(yo[:], yf[:], semT_sb[:, :, ncol],
                                        op=ALU.add)
                for m2 in range(2):
                    nc.sync.dma_start(
                        outT_r[:, 4 * m2:4 * m2 + 4, ncol],
                        yo[:, 4 * m2:4 * m2 + 4, :])
        es_fw.close()

    nc.compile()
    return nc


_NC_CACHE = {}


def _get_nc(debug_outs=False):
    key = bool(debug_outs)
    if key not in _NC_CACHE:
        _NC_CACHE[key] = build_nc(debug_outs)
    return _NC_CACHE[key]


def make_in_maps(inputs):
    """Host-side shard + layout prep. inputs: dict of full np arrays."""
    import ml_dtypes
    f8 = ml_dtypes.float8_e4m3
    f32 = np.float32
    sem = np.asarray(inputs["sem"], f32)
    pro = np.asarray(inputs["pro"], f32)

    def cols(v, nm):
        return np.ascontiguousarray(np.asarray(v, f32).reshape(nm, P).T)

    def wlay(wT, nkt, nm):
        # [in=nkt*128, out=nm*128] -> [128p, nkt, nm, 128c]
        return np.ascontiguousarray(
            wT.reshape(nkt, P, nm, P).transpose(1, 0, 2, 3))

    WqT = np.asarray(inputs["Wq"], f32).T * WS
    WkT = np.asarray(inputs["Wk"], f32).T * WS
    WvT = np.asarray(inputs["Wv"], f32).T * WS
    WoT = np.asarray(inputs["Wo"], f32).T * WS

    W1 = np.asarray(inputs["W1"], f32)
    W2 = np.asarray(inputs["W2"], f32)
    mw1 = np.maximum(np.abs(W1).mean(), 1e-5)
    mw2 = np.maximum(np.abs(W2).mean(), 1e-5)
    w1s = np.clip(np.round(W1 / mw1), -1, 1)          # [DF, DS]
    w2s = np.clip(np.round(W2 / mw2), -1, 1)          # [DS, DF]

    alpha = np.asarray(inputs["alpha"], f32)
    beta = np.asarray(inputs["beta"], f32)
    acos_v = 2.0 * alpha * mw1
    rb2 = 1.0 / (2.0 * (beta + 1e-9) * mw1)           # positive snake const
    ybias_v = (mw1 * mw2) * (w2s @ rb2)               # [DS]
    obias_v = np.asarray(inputs["bo"], f32) + \
        np.asarray(inputs["Wo"], f32) @ np.asarray(inputs["bv"], f32)

    common = {
        "gsem": cols(inputs["g_sem"], M_SEM),
        "gpro": cols(inputs["g_pro"], M_PRO),
        "gff": cols(inputs["g_ff"], M_SEM),
        "bq": cols(inputs["bq"], M_SEM),
        "bk": cols(inputs["bk"], M_SEM),
        "obias": cols(obias_v, M_SEM),
        "ybias": cols(ybias_v, M_SEM),
        "acos": cols(acos_v, M_FF),
        "nrb2": cols(-rb2, M_FF),
        "dqs": np.full((P, 1), mw1 * mw2, f32),
        "wq": wlay(WqT, M_SEM, M_SEM).astype(f8),
        "wk": wlay(WkT, M_PRO, M_SEM).astype(f8),
        "wv": np.ascontiguousarray(
            WvT.reshape(M_PRO, P, DS).transpose(1, 0, 2)).astype(f8),
        "wo": wlay(WoT, M_SEM, M_SEM).astype(f8),
        "w1q": wlay(np.ascontiguousarray(w1s.T), M_SEM, M_FF).astype(f8),
        "w2q": wlay(np.ascontiguousarray(w2s.T), M_FF, M_SEM).astype(f8),
    }

    in_maps = []
    for c in range(N_CORES):
        b, half = c // 2, c % 2
        m = dict(common)
        m["semT"] = np.ascontiguousarray(sem[b, half * TOK:(half + 1) * TOK, :].T)
        m["proT"] = np.ascontiguousarray(pro[b].T).astype(ml_dtypes.bfloat16)
        in_maps.append(m)
    return in_maps


def assemble_out(results):
    out = np.empty((B, S, DS), np.float32)
    for c in range(N_CORES):
        b, half = c // 2, c % 2
        out[b, half * TOK:(half + 1) * TOK, :] = results[c]["outT"].T
    return out


def kernel(**inputs):
    nc = _get_nc()
    in_maps = make_in_maps(inputs)
    res = run_bass_kernel_spmd(nc, in_maps, core_ids=list(range(N_CORES)))
    return assemble_out(res.results)


# revision 25
# speedup vs baseline: 1.1702x; 1.1702x over previous
"""Trainium2 Bass kernel for nn_CrossAttentionFusion (cross-attention + BitLinear FFN).

Sharding: 8 cores = 4 batches x 2 sequence-halves (data parallel, no collectives).
Each core owns 1024 query tokens; computes K/V for its batch's full 2048 tokens.

v2: fp8 DoubleRow matmuls everywhere (QKV/O/PV/FFN), host-side ternarization of
W1/W2 (shipped as fp8 +-1/0), softmax denominator computed on the PE via a
ones-matmul, act_quant realized as a direct fp8 cast.
"""
import math
import numpy as np
from contextlib import ExitStack

import concourse.bass as bass
import concourse.bass_isa as bass_isa
import concourse.tile as tile
from concourse import bacc, mybir
from concourse.bass_utils import run_bass_kernel_spmd

F32 = mybir.dt.float32
F32R = mybir.dt.float32r
BF16 = mybir.dt.bfloat16
FP8 = mybir.dt.float8e4
AF = mybir.ActivationFunctionType
ALU = mybir.AluOpType
DR = mybir.MatmulPerfMode.DoubleRow

B, S, DS, DP, H = 4, 2048, 1024, 512, 8
DF = 4 * DS
HD = DS // H          # 128
TOK = 1024            # query tokens per core
N_CORES = 8
EPS = 1e-6
QK_SCALE = 1.0 / math.sqrt(HD)
HALF_PI = math.pi / 2.0
WS = 32.0             # fp8 scale for Wq/Wk/Wv/Wo
CTX_S = 64.0          # fp8 scale for ctx (folded into 1/den via ones=1/64)

P = 128
M_SEM = DS // P       # 8
M_PRO = DP // P       # 4
M_FF = DF // P        # 32
NT_K = S // P         # 16


def build_nc(debug_outs=False):
    nc = bacc.Bacc("TRN2", target_bir_lowering=False, debug=False, num_devices=N_CORES)

    semT = nc.dram_tensor("semT", [DS, TOK], F32, kind="ExternalInput").ap()
    proT = nc.dram_tensor("proT", [DP, S], BF16, kind="ExternalInput").ap()
    wq = nc.dram_tensor("wq", [P, M_SEM, M_SEM, P], FP8, kind="ExternalInput").ap()
    wk = nc.dram_tensor("wk", [P, M_PRO, M_SEM, P], FP8, kind="ExternalInput").ap()
    wv = nc.dram_tensor("wv", [P, M_PRO, DS], FP8, kind="ExternalInput").ap()
    wo = nc.dram_tensor("wo", [P, M_SEM, M_SEM, P], FP8, kind="ExternalInput").ap()
    w1q = nc.dram_tensor("w1q", [P, M_SEM, M_FF, P], FP8, kind="ExternalInput").ap()
    w2q = nc.dram_tensor("w2q", [P, M_FF, M_SEM, P], FP8, kind="ExternalInput").ap()
    gsem = nc.dram_tensor("gsem", [P, M_SEM], F32, kind="ExternalInput").ap()
    gpro = nc.dram_tensor("gpro", [P, M_PRO], F32, kind="ExternalInput").ap()
    gff = nc.dram_tensor("gff", [P, M_SEM], F32, kind="ExternalInput").ap()
    bq = nc.dram_tensor("bq", [P, M_SEM], F32, kind="ExternalInput").ap()
    bk = nc.dram_tensor("bk", [P, M_SEM], F32, kind="ExternalInput").ap()
    obias = nc.dram_tensor("obias", [P, M_SEM], F32, kind="ExternalInput").ap()
    ybias = nc.dram_tensor("ybias", [P, M_SEM], F32, kind="ExternalInput").ap()
    acos = nc.dram_tensor("acos", [P, M_FF], F32, kind="ExternalInput").ap()
    nrb2 = nc.dram_tensor("nrb2", [P, M_FF], F32, kind="ExternalInput").ap()
    dqs = nc.dram_tensor("dqs", [P, 1], F32, kind="ExternalInput").ap()
    outT = nc.dram_tensor("outT", [DS, TOK], F32, kind="ExternalOutput").ap()

    semT_r = semT.rearrange("(m p) t -> p m t", p=P)
    proT_r = proT.rearrange("(m p) t -> p m t", p=P)
    outT_r = outT.rearrange("(m p) t -> p m t", p=P)

    with tile.TileContext(nc) as tc, ExitStack() as top:
        persist = top.enter_context(tc.tile_pool(name="persist", bufs=1))

        ones = persist.tile([P, 1], BF16)
        nc.vector.memset(ones[:], 1.0)
        ones_row = persist.tile([1, P], BF16)
        nc.vector.memset(ones_row[:], 1.0)
        ones2_f8 = persist.tile([P, 2, P], FP8)
        nc.vector.memset(ones2_f8[:], 1.0 / CTX_S)
        eps_t = persist.tile([1, 1], F32)
        nc.vector.memset(eps_t[:], EPS)
        hpi_t = persist.tile([P, 1], F32)
        nc.vector.memset(hpi_t[:], HALF_PI)

        gsem_sb = persist.tile([P, M_SEM], F32)
        gpro_sb = persist.tile([P, M_PRO], F32)
        gff_sb = persist.tile([P, M_SEM], F32)
        bq_sb = persist.tile([P, M_SEM], F32)
        bk_sb = persist.tile([P, M_SEM], F32)
        obias_sb = persist.tile([P, M_SEM], F32)
        ybias_sb = persist.tile([P, M_SEM], F32)
        acos_sb = persist.tile([P, M_FF], F32)
        nrb2_sb = persist.tile([P, M_FF], F32)
        dqs_sb = persist.tile([P, 1], F32)
        for ap_d, t in [(gsem, gsem_sb), (gpro, gpro_sb), (gff, gff_sb),
                        (bq, bq_sb), (bk, bk_sb), (obias, obias_sb),
                        (ybias, ybias_sb), (acos, acos_sb), (nrb2, nrb2_sb),
                        (dqs, dqs_sb)]:
            nc.sync.dma_start(t[:], ap_d[:])

        # persistent big tensors
        semT_sb = persist.tile([P, M_SEM, TOK], F32)      # becomes semout in place
        es_att = ExitStack()
        patt_io = es_att.enter_context(tc.tile_pool(name="patt_io", bufs=1))
        q_sb = patt_io.tile([P, M_SEM, TOK], FP8)
        k_sb = patt_io.tile([P, M_SEM, S], FP8)
        v_sb = patt_io.tile([P, NT_K, DS], FP8)
        ctx_n = [persist.tile([P, M_SEM, 512], FP8, name=f"ctx{n}")
                 for n in range(2)]

        for m2 in range(4):
            nc.sync.dma_start(semT_sb[:, 2 * m2:2 * m2 + 2, :],
                              semT_r[:, 2 * m2:2 * m2 + 2, :])

        # ---------- input norms -> semn/pron in fp8 ----------
        es_norm = ExitStack()
        pnorm = es_norm.enter_context(tc.tile_pool(name="pnorm", bufs=1))
        semn = pnorm.tile([P, M_SEM, TOK], FP8)
        pron = pnorm.tile([P, M_PRO, S], FP8)
        ps_nrm = es_norm.enter_context(
            tc.tile_pool(name="ps_nrm", bufs=2, space="PSUM"))

        with tc.tile_pool(name="pnsc", bufs=1) as pnsc:
            proT_sb = pnsc.tile([P, M_PRO, S], BF16)
            for m in range(M_PRO):
                nc.sync.dma_start(proT_sb[:, m, :], proT_r[:, m, :])

            def rmsnorm(x_sb, nm, T, g, out_f8):
                D = nm * P
                sq = pnsc.tile([P, nm * T], BF16, tag="sq", bufs=1, name="sq")
                xf = x_sb[:].rearrange("p a b -> p (a b)")
                half = nm * T // 2
                nc.scalar.activation(sq[:, :half], xf[:, :half], AF.Square)
                nc.scalar.activation(sq[:, half:], xf[:, half:], AF.Square)
                rs_row = pnsc.tile([1, T], F32, tag=f"rs{nm}", bufs=1)
                for ch in range(T // 512):
                    pst = ps_nrm.tile([P, 512], F32, tag="nrm")
                    ps = pst[0:1, :]
                    for m in range(nm):
                        nc.tensor.matmul(
                            ps[:], ones[:],
                            sq[:, m * T + ch * 512:m * T + (ch + 1) * 512],
                            start=(m == 0), stop=(m == nm - 1))
                    nc.scalar.activation(rs_row[:, ch * 512:(ch + 1) * 512],
                                         ps[:], AF.Ln, bias=eps_t[:],
                                         scale=1.0 / D)
                nc.scalar.activation(rs_row[:], rs_row[:], AF.Exp, scale=-0.5)
                rs_bc = pnsc.tile([P, T], F32, tag=f"rsbc{nm}", bufs=1)
                nc.gpsimd.partition_broadcast(rs_bc[:], rs_row[:])
                for m in range(nm):
                    nc.vector.scalar_tensor_tensor(
                        out=out_f8[:, m, :], in0=x_sb[:, m, :],
                        scalar=g[:, m:m + 1], in1=rs_bc[:],
                        op0=ALU.mult, op1=ALU.mult)

            rmsnorm(semT_sb, M_SEM, TOK, gsem_sb, semn)
            rmsnorm(proT_sb, M_PRO, S, gpro_sb, pron)

        # ---------- QKV projections (fp8 DoubleRow) ----------
        es_qw = ExitStack()
        pqw = es_qw.enter_context(tc.tile_pool(name="pqw", bufs=1, side="right"))
        wq_sb = pqw.tile([P, M_SEM, M_SEM, P], FP8)
        wk_sb = pqw.tile([P, M_PRO, M_SEM, P], FP8)
        wv_sb = pqw.tile([P, M_PRO, DS], FP8)
        nc.sync.dma_start(wq_sb[:], wq[:])
        nc.sync.dma_start(wk_sb[:], wk[:])
        nc.sync.dma_start(wv_sb[:], wv[:])

        with tc.tile_pool(name="ps_mm", bufs=2, space="PSUM") as ps_mm:
            # Q: out q_sb[:, m, 0:1024]
            for m in range(M_SEM):
                ps = ps_mm.tile([P, 1024], F32, tag="mm")
                for n2 in range(2):
                    for kk in range(4):
                        nc.tensor.matmul(
                            ps[:, n2 * 512:(n2 + 1) * 512],
                            wq_sb[:, 2 * kk:2 * kk + 2, m, :],
                            semn[:, 2 * kk:2 * kk + 2, n2 * 512:(n2 + 1) * 512],
                            start=(kk == 0), stop=(kk == 3), perf_mode=DR)
                nc.vector.tensor_scalar(q_sb[:, m, :], ps[:], 1.0 / WS,
                                        bq_sb[:, m:m + 1], ALU.mult, ALU.add)
            # K: out k_sb[:, m, 0:2048]
            for m in range(M_SEM):
                for c2 in range(2):
                    ps = ps_mm.tile([P, 1024], F32, tag="mm")
                    for n2 in range(2):
                        col = (c2 * 2 + n2) * 512
                        for kk in range(2):
                            nc.tensor.matmul(
                                ps[:, n2 * 512:(n2 + 1) * 512],
                                wk_sb[:, 2 * kk:2 * kk + 2, m, :],
                                pron[:, 2 * kk:2 * kk + 2, col:col + 512],
                                start=(kk == 0), stop=(kk == 1), perf_mode=DR)
                    nc.scalar.activation(k_sb[:, m, c2 * 1024:(c2 + 1) * 1024],
                                         ps[:], AF.Identity, scale=1.0 / WS,
                                         bias=bk_sb[:, m:m + 1])
            # V (transposed, kpos-major): out v_sb[:, mt, 0:1024]
            for mt in range(NT_K):
                ps = ps_mm.tile([P, 1024], F32, tag="mm")
                for n2 in range(2):
                    for kk in range(2):
                        nc.tensor.matmul(
                            ps[:, n2 * 512:(n2 + 1) * 512],
                            pron[:, 2 * kk:2 * kk + 2, mt * P:(mt + 1) * P],
                            wv_sb[:, 2 * kk:2 * kk + 2, n2 * 512:(n2 + 1) * 512],
                            start=(kk == 0), stop=(kk == 1), perf_mode=DR)
                nc.vector.tensor_scalar(v_sb[:, mt, :], ps[:], 1.0 / WS, None,
                                        ALU.mult)
        es_norm.close()
        es_qw.close()

        # FFN weights arrive during attention
        es_fw = ExitStack()
        pfw = es_fw.enter_context(tc.tile_pool(name="pfw", bufs=1, side="right"))
        w1q_sb = pfw.tile([P, M_SEM, M_FF, P], FP8)
        w2q_sb = pfw.tile([P, M_FF, M_SEM, P], FP8)
        for c4 in range(4):
            nc.sync.dma_start(w1q_sb[:, 2 * c4:2 * c4 + 2], w1q[:, 2 * c4:2 * c4 + 2])
        for c4 in range(4):
            nc.sync.dma_start(w2q_sb[:, 8 * c4:8 * c4 + 8], w2q[:, 8 * c4:8 * c4 + 8])
        wo_sb = pfw.tile([P, M_SEM, M_SEM, P], FP8)
        nc.sync.dma_start(wo_sb[:], wo[:])

        # ---------- attention ----------
        with tc.tile_pool(name="pattn", bufs=1) as pattn, \
             tc.tile_pool(name="ps_att", bufs=1, space="PSUM") as ps_att:
            for n in range(2):
                for h in range(H):
                    pt = pattn.tile([P, NT_K, 512], FP8, tag="pt", bufs=2)
                    den = ps_att.tile([P, 512], F32, tag="den", bufs=2)
                    cps = ps_att.tile([P, 512], F32, tag="pv", bufs=2)
                    for kp in range(8):
                        sc = ps_att.tile([P, 1024], F32, tag="sc", bufs=2)
                        nc.tensor.matmul(
                            sc[:, 0:512],
                            k_sb[:, h, (2 * kp) * P:(2 * kp + 1) * P],
                            q_sb[:, h, n * 512:(n + 1) * 512],
                            start=True, stop=True)
                        nc.tensor.matmul(
                            sc[:, 512:1024],
                            k_sb[:, h, (2 * kp + 1) * P:(2 * kp + 2) * P],
                            q_sb[:, h, n * 512:(n + 1) * 512],
                            start=True, stop=True)
                        nc.scalar.activation(
                            pt[:, 2 * kp:2 * kp + 2, :].rearrange(
                                "p a b -> p (a b)"),
                            sc[:], AF.Exp, scale=QK_SCALE)
                        nc.tensor.matmul(den[:], ones2_f8[:],
                                         pt[:, 2 * kp:2 * kp + 2, :],
                                         start=(kp == 0), stop=(kp == 7),
                                         perf_mode=DR)
                        nc.tensor.matmul(cps[:],
                                         v_sb[:, 2 * kp:2 * kp + 2,
                                              h * P:(h + 1) * P],
                                         pt[:, 2 * kp:2 * kp + 2, :],
                                         start=(kp == 0), stop=(kp == 7),
                                         perf_mode=DR)
                    bc_sb = pattn.tile([P, 512], F32, tag="bcsb", bufs=2)
                    nc.vector.reciprocal_approx_fast(bc_sb[:], den[:])
                    nc.vector.tensor_tensor(ctx_n[n][:, h, :], cps[:], bc_sb[:],
                                            op=ALU.mult)
        es_att.close()

        # ---------- out-proj + ffn-norm + FFN (per half) ----------
        with tc.tile_pool(name="pff", bufs=1) as pff, \
             tc.tile_pool(name="ps_ff", bufs=2, space="PSUM") as ps_ff:
            xq_n = [pff.tile([P, M_SEM, 512], FP8, name=f"xq{n}")
                    for n in range(2)]
            h2_n = [pff.tile([P, M_FF, 512], FP8, name=f"h2{n}")
                    for n in range(2)]
            for n in range(2):
                ncol = slice(n * 512, (n + 1) * 512)
                # out-proj
                t_o = pff.tile([P, M_SEM, 512], BF16, tag="t_o", bufs=1)
                for m in range(M_SEM):
                    ps = ps_ff.tile([P, 512], F32, tag="mmf")
                    for kk in range(4):
                        nc.tensor.matmul(ps[:], wo_sb[:, 2 * kk:2 * kk + 2, m, :],
                                         ctx_n[n][:, 2 * kk:2 * kk + 2, :],
                                         start=(kk == 0), stop=(kk == 3),
                                         perf_mode=DR)
                    nc.vector.tensor_scalar(t_o[:, m, :], ps[:],
                                            1.0 / (WS * CTX_S),
                                            obias_sb[:, m:m + 1],
                                            ALU.mult, ALU.add)
                semo = semT_sb[:, :, ncol]
                nc.vector.tensor_tensor(semo, t_o[:], semo, op=ALU.add)

                # ffn norm -> xq (fp8)
                sqf = pff.tile([P, M_SEM, 512], BF16, tag="sqf", bufs=1)
                nc.gpsimd.tensor_tensor(sqf[:], semo, semo, op=ALU.mult)
                pst = ps_ff.tile([P, 512], F32, tag="nrmf")
                psr = pst[0:1, :]
                for m in range(M_SEM):
                    nc.tensor.matmul(psr[:], ones[:], sqf[:, m, :],
                                     start=(m == 0), stop=(m == M_SEM - 1))
                rsf = pff.tile([1, 512], F32, tag="rsf", bufs=1)
                nc.scalar.activation(rsf[:], psr[:], AF.Ln, bias=eps_t[:],
                                     scale=1.0 / DS)
                nc.scalar.activation(rsf[:], rsf[:], AF.Exp, scale=-0.5)
                rsbc = pff.tile([P, 512], F32, tag="rsbc", bufs=1)
                nc.gpsimd.partition_broadcast(rsbc[:], rsf[:])
                for m in range(M_SEM):
                    nc.vector.scalar_tensor_tensor(
                        out=xq_n[n][:, m, :], in0=semT_sb[:, m, ncol],
                        scalar=gff_sb[:, m:m + 1], in1=rsbc[:],
                        op0=ALU.mult, op1=ALU.mult)

                # FFN1 + snake
                for m in range(M_FF):
                    ps = ps_ff.tile([P, 512], F32, tag="mmf")
                    for kk in range(4):
                        nc.tensor.matmul(ps[:], w1q_sb[:, 2 * kk:2 * kk + 2, m, :],
                                         xq_n[n][:, 2 * kk:2 * kk + 2, :],
                                         start=(kk == 0), stop=(kk == 3),
                                         perf_mode=DR)
                    sn = pff.tile([P, 512], BF16, tag="sn", bufs=3)
                    nc.scalar.activation(sn[:], ps[:], AF.Sin, bias=hpi_t[:],
                                         scale=acos_sb[:, m:m + 1])
                    nc.vector.scalar_tensor_tensor(
                        out=h2_n[n][:, m, :], in0=sn[:],
                        scalar=nrb2_sb[:, m:m + 1], in1=ps[:],
                        op0=ALU.mult, op1=ALU.add)

            # FFN2 (both halves; h2 ready per half in order)
            for n in range(2):
                ncol = slice(n * 512, (n + 1) * 512)
                yf = pff.tile([P, M_SEM, 512], BF16, tag="yf", bufs=1)
                for m in range(M_SEM):
                    ps = ps_ff.tile([P, 512], F32, tag="mmf")
                    for kk in range(16):
                        nc.tensor.matmul(ps[:], w2q_sb[:, 2 * kk:2 * kk + 2, m, :],
                                         h2_n[n][:, 2 * kk:2 * kk + 2, :],
                                         start=(kk == 0), stop=(kk == 15),
                                         perf_mode=DR)
                    nc.vector.tensor_scalar(yf[:, m, :], ps[:], dqs_sb[:],
                                            ybias_sb[:, m:m + 1],
                                            ALU.mult, ALU.add)
                yo = pff.tile([P, M_SEM, 512], F32, tag="yo", bufs=1)
                nc.vector.tensor_tensor(yo[:], yf[:], semT_sb[:, :, ncol],
                                        op=ALU.add)
                for m2 in range(2):
                    nc.sync.dma_start(
                        outT_r[:, 4 * m2:4 * m2 + 4, ncol],
                        yo[:, 4 * m2:4 * m2 + 4, :])
        es_fw.close()

    nc.compile()
    return nc


_NC_CACHE = {}


def _get_nc(debug_outs=False):
    key = bool(debug_outs)
    if key not in _NC_CACHE:
        _NC_CACHE[key] = build_nc(debug_outs)
    return _NC_CACHE[key]


def make_in_maps(inputs):
    """Host-side shard + layout prep. inputs: dict of full np arrays."""
    import ml_dtypes
    f8 = ml_dtypes.float8_e4m3
    f32 = np.float32
    sem = np.asarray(inputs["sem"], f32)
    pro = np.asarray(inputs["pro"], f32)

    def cols(v, nm):
        return np.ascontiguousarray(np.asarray(v, f32).reshape(nm, P).T)

    def wlay(wT, nkt, nm):
        # [in=nkt*128, out=nm*128] -> [128p, nkt, nm, 128c]
        return np.ascontiguousarray(
            wT.reshape(nkt, P, nm, P).transpose(1, 0, 2, 3))

    WqT = np.asarray(inputs["Wq"], f32).T * WS
    WkT = np.asarray(inputs["Wk"], f32).T * WS
    WvT = np.asarray(inputs["Wv"], f32).T * WS
    WoT = np.asarray(inputs["Wo"], f32).T * WS

    W1 = np.asarray(inputs["W1"], f32)
    W2 = np.asarray(inputs["W2"], f32)
    mw1 = np.maximum(np.abs(W1).mean(), 1e-5)
    mw2 = np.maximum(np.abs(W2).mean(), 1e-5)
    w1s = np.clip(np.round(W1 / mw1), -1, 1)          # [DF, DS]
    w2s = np.clip(np.round(W2 / mw2), -1, 1)          # [DS, DF]

    alpha = np.asarray(inputs["alpha"], f32)
    beta = np.asarray(inputs["beta"], f32)
    acos_v = 2.0 * alpha * mw1
    rb2 = 1.0 / (2.0 * (beta + 1e-9) * mw1)           # positive snake const
    ybias_v = (mw1 * mw2) * (w2s @ rb2)               # [DS]
    obias_v = np.asarray(inputs["bo"], f32) + \
        np.asarray(inputs["Wo"], f32) @ np.asarray(inputs["bv"], f32)

    common = {
        "gsem": cols(inputs["g_sem"], M_SEM),
        "gpro": cols(inputs["g_pro"], M_PRO),
        "gff": cols(inputs["g_ff"], M_SEM),
        "bq": cols(inputs["bq"], M_SEM),
        "bk": cols(inputs["bk"], M_SEM),
        "obias": cols(obias_v, M_SEM),
        "ybias": cols(ybias_v, M_SEM),
        "acos": cols(acos_v, M_FF),
        "nrb2": cols(-rb2, M_FF),
        "dqs": np.full((P, 1), mw1 * mw2, f32),
        "wq": wlay(WqT, M_SEM, M_SEM).astype(f8),
        "wk": wlay(WkT, M_PRO, M_SEM).astype(f8),
        "wv": np.ascontiguousarray(
            WvT.reshape(M_PRO, P, DS).transpose(1, 0, 2)).astype(f8),
        "wo": wlay(WoT, M_SEM, M_SEM).astype(f8),
        "w1q": wlay(np.ascontiguousarray(w1s.T), M_SEM, M_FF).astype(f8),
        "w2q": wlay(np.ascontiguousarray(w2s.T), M_FF, M_SEM).astype(f8),
    }

    in_maps = []
    for c in range(N_CORES):
        b, half = c // 2, c % 2
        m = dict(common)
        m["semT"] = np.ascontiguousarray(sem[b, half * TOK:(half + 1) * TOK, :].T)
        m["proT"] = np.ascontiguousarray(pro[b].T).astype(ml_dtypes.bfloat16)
        in_maps.append(m)
    return in_maps


def assemble_out(results):
    out = np.empty((B, S, DS), np.float32)
    for c in range(N_CORES):
        b, half = c // 2, c % 2
        out[b, half * TOK:(half + 1) * TOK, :] = results[c]["outT"].T
    return out


def kernel(**inputs):
    nc = _get_nc()
    in_maps = make_in_maps(inputs)
    res = run_bass_kernel_spmd(nc, in_maps, core_ids=list(range(N_CORES)))
    return assemble_out(res.results)


# revision 31
# speedup vs baseline: 1.2993x; 1.1103x over previous
"""Trainium2 Bass kernel for nn_CrossAttentionFusion (cross-attention + BitLinear FFN).

Sharding: 8 cores = 4 batches x 2 sequence-halves (data parallel, no collectives).
Each core owns 1024 query tokens; computes K/V for its batch's full 2048 tokens.

v2: fp8 DoubleRow matmuls everywhere (QKV/O/PV/FFN), host-side ternarization of
W1/W2 (shipped as fp8 +-1/0), softmax denominator computed on the PE via a
ones-matmul, act_quant realized as a direct fp8 cast.
"""
import math
import numpy as np
from contextlib import ExitStack

import concourse.bass as bass
import concourse.bass_isa as bass_isa
import concourse.tile as tile
from concourse import bacc, mybir
from concourse.bass_utils import run_bass_kernel_spmd

F32 = mybir.dt.float32
F32R = mybir.dt.float32r
BF16 = mybir.dt.bfloat16
FP8 = mybir.dt.float8e4
AF = mybir.ActivationFunctionType
ALU = mybir.AluOpType
DR = mybir.MatmulPerfMode.DoubleRow

B, S, DS, DP, H = 4, 2048, 1024, 512, 8
DF = 4 * DS
HD = DS // H          # 128
TOK = 1024            # query tokens per core
N_CORES = 8
EPS = 1e-6
QK_SCALE = 1.0 / math.sqrt(HD)
HALF_PI = math.pi / 2.0
WS = 32.0             # fp8 scale for Wq/Wk/Wv/Wo
CTX_S = 64.0          # fp8 scale for ctx (folded into 1/den via ones=1/64)

P = 128
M_SEM = DS // P       # 8
M_PRO = DP // P       # 4
M_FF = DF // P        # 32
NT_K = S // P         # 16


def build_nc(debug_outs=False):
    nc = bacc.Bacc("TRN2", target_bir_lowering=False, debug=False, num_devices=N_CORES)

    semT = nc.dram_tensor("semT", [DS, TOK], F32, kind="ExternalInput").ap()
    proT = nc.dram_tensor("proT", [DP, S], BF16, kind="ExternalInput").ap()
    wq = nc.dram_tensor("wq", [P, M_SEM, M_SEM, P], FP8, kind="ExternalInput").ap()
    wk = nc.dram_tensor("wk", [P, M_PRO, M_SEM, P], FP8, kind="ExternalInput").ap()
    wv = nc.dram_tensor("wv", [P, M_PRO, DS], FP8, kind="ExternalInput").ap()
    wo = nc.dram_tensor("wo", [P, M_SEM, M_SEM, P], FP8, kind="ExternalInput").ap()
    w1q = nc.dram_tensor("w1q", [P, M_SEM, M_FF, P], FP8, kind="ExternalInput").ap()
    w2q = nc.dram_tensor("w2q", [P, M_FF, M_SEM, P], FP8, kind="ExternalInput").ap()
    gsem = nc.dram_tensor("gsem", [P, M_SEM], F32, kind="ExternalInput").ap()
    gpro = nc.dram_tensor("gpro", [P, M_PRO], F32, kind="ExternalInput").ap()
    gff = nc.dram_tensor("gff", [P, M_SEM], F32, kind="ExternalInput").ap()
    bq = nc.dram_tensor("bq", [P, M_SEM], F32, kind="ExternalInput").ap()
    bk = nc.dram_tensor("bk", [P, M_SEM], F32, kind="ExternalInput").ap()
    obias = nc.dram_tensor("obias", [P, M_SEM], F32, kind="ExternalInput").ap()
    ybias = nc.dram_tensor("ybias", [P, M_SEM], F32, kind="ExternalInput").ap()
    acos = nc.dram_tensor("acos", [P, M_FF], F32, kind="ExternalInput").ap()
    nrb2 = nc.dram_tensor("nrb2", [P, M_FF], F32, kind="ExternalInput").ap()
    dqs = nc.dram_tensor("dqs", [P, 1], F32, kind="ExternalInput").ap()
    outT = nc.dram_tensor("outT", [DS, TOK], F32, kind="ExternalOutput").ap()

    semT_r = semT.rearrange("(m p) t -> p m t", p=P)
    proT_r = proT.rearrange("(m p) t -> p m t", p=P)
    outT_r = outT.rearrange("(m p) t -> p m t", p=P)

    with tile.TileContext(nc) as tc, ExitStack() as top:
        persist = top.enter_context(tc.tile_pool(name="persist", bufs=1))

        ones = persist.tile([P, 1], BF16)
        nc.vector.memset(ones[:], 1.0)
        ones_full = persist.tile([P, P], BF16)
        nc.vector.memset(ones_full[:], 1.0)
        ones_row = persist.tile([1, P], BF16)
        nc.vector.memset(ones_row[:], 1.0)
        ones2_f8 = persist.tile([P, 2, P], FP8)
        nc.vector.memset(ones2_f8[:], 1.0 / CTX_S)
        eps_t = persist.tile([P, 1], F32)
        nc.vector.memset(eps_t[:], EPS)
        hpi_t = persist.tile([P, 1], F32)
        nc.vector.memset(hpi_t[:], HALF_PI)

        gsem_sb = persist.tile([P, M_SEM], F32)
        gpro_sb = persist.tile([P, M_PRO], F32)
        gff_sb = persist.tile([P, M_SEM], F32)
        bq_sb = persist.tile([P, M_SEM], F32)
        bk_sb = persist.tile([P, M_SEM], F32)
        obias_sb = persist.tile([P, M_SEM], F32)
        ybias_sb = persist.tile([P, M_SEM], F32)
        acos_sb = persist.tile([P, M_FF], F32)
        nrb2_sb = persist.tile([P, M_FF], F32)
        dqs_sb = persist.tile([P, 1], F32)
        for ap_d, t in [(gsem, gsem_sb), (gpro, gpro_sb), (gff, gff_sb),
                        (bq, bq_sb), (bk, bk_sb), (obias, obias_sb),
                        (ybias, ybias_sb), (acos, acos_sb), (nrb2, nrb2_sb),
                        (dqs, dqs_sb)]:
            nc.sync.dma_start(t[:], ap_d[:])

        # persistent big tensors
        semT_sb = persist.tile([P, M_SEM, TOK], F32)      # becomes semout in place
        es_att = ExitStack()
        patt_io = es_att.enter_context(tc.tile_pool(name="patt_io", bufs=1))
        q_sb = patt_io.tile([P, M_SEM, TOK], FP8)
        k_sb = patt_io.tile([P, M_SEM, S], FP8)
        v_sb = patt_io.tile([P, NT_K, DS], FP8)
        ctx_n = [persist.tile([P, M_SEM, 512], FP8, name=f"ctx{n}")
                 for n in range(2)]

        for m2 in range(4):
            nc.sync.dma_start(semT_sb[:, 2 * m2:2 * m2 + 2, :],
                              semT_r[:, 2 * m2:2 * m2 + 2, :])

        # ---------- input norms -> semn/pron in fp8 ----------
        es_norm = ExitStack()
        pnorm = es_norm.enter_context(tc.tile_pool(name="pnorm", bufs=1))
        semn = pnorm.tile([P, M_SEM, TOK], FP8)
        pron = pnorm.tile([P, M_PRO, S], FP8)
        ps_nrm = es_norm.enter_context(
            tc.tile_pool(name="ps_nrm", bufs=2, space="PSUM"))

        with tc.tile_pool(name="pnsc", bufs=1) as pnsc:
            proT_sb = pnsc.tile([P, M_PRO, S], BF16)
            for m in range(M_PRO):
                nc.sync.dma_start(proT_sb[:, m, :], proT_r[:, m, :])

            def rmsnorm(x_sb, nm, T, g, out_f8):
                D = nm * P
                sq = pnsc.tile([P, nm * T], BF16, tag="sq", bufs=1, name="sq")
                xf = x_sb[:].rearrange("p a b -> p (a b)")
                half = nm * T // 2
                nc.scalar.activation(sq[:, :half], xf[:, :half], AF.Square)
                nc.scalar.activation(sq[:, half:], xf[:, half:], AF.Square)
                rs_bc = pnsc.tile([P, T], F32, tag=f"rsbc{nm}", bufs=1)
                for ch in range(T // 512):
                    pst = ps_nrm.tile([P, 512], F32, tag="nrm")
                    for m in range(nm):
                        nc.tensor.matmul(
                            pst[:], ones_full[:],
                            sq[:, m * T + ch * 512:m * T + (ch + 1) * 512],
                            start=(m == 0), stop=(m == nm - 1))
                    nc.scalar.activation(rs_bc[:, ch * 512:(ch + 1) * 512],
                                         pst[:], AF.Ln, bias=eps_t[:],
                                         scale=1.0 / D)
                nc.scalar.activation(rs_bc[:], rs_bc[:], AF.Exp, scale=-0.5)
                for m in range(nm):
                    nc.vector.scalar_tensor_tensor(
                        out=out_f8[:, m, :], in0=x_sb[:, m, :],
                        scalar=g[:, m:m + 1], in1=rs_bc[:],
                        op0=ALU.mult, op1=ALU.mult)

            rmsnorm(semT_sb, M_SEM, TOK, gsem_sb, semn)
            rmsnorm(proT_sb, M_PRO, S, gpro_sb, pron)

        # ---------- QKV projections (fp8 DoubleRow) ----------
        es_qw = ExitStack()
        pqw = es_qw.enter_context(tc.tile_pool(name="pqw", bufs=1, side="right"))
        wq_sb = pqw.tile([P, M_SEM, M_SEM, P], FP8)
        wk_sb = pqw.tile([P, M_PRO, M_SEM, P], FP8)
        wv_sb = pqw.tile([P, M_PRO, DS], FP8)
        nc.sync.dma_start(wq_sb[:], wq[:])
        nc.sync.dma_start(wk_sb[:], wk[:])
        nc.sync.dma_start(wv_sb[:], wv[:])

        with tc.tile_pool(name="ps_mm", bufs=2, space="PSUM") as ps_mm:
            # Q: out q_sb[:, m, 0:1024]
            for m in range(M_SEM):
                ps = ps_mm.tile([P, 1024], F32, tag="mm")
                for n2 in range(2):
                    for kk in range(4):
                        nc.tensor.matmul(
                            ps[:, n2 * 512:(n2 + 1) * 512],
                            wq_sb[:, 2 * kk:2 * kk + 2, m, :],
                            semn[:, 2 * kk:2 * kk + 2, n2 * 512:(n2 + 1) * 512],
                            start=(kk == 0), stop=(kk == 3), perf_mode=DR)
                nc.vector.tensor_scalar(q_sb[:, m, :], ps[:], 1.0 / WS,
                                        bq_sb[:, m:m + 1], ALU.mult, ALU.add)
            # K: out k_sb[:, m, 0:2048]
            for m in range(M_SEM):
                for c2 in range(2):
                    ps = ps_mm.tile([P, 1024], F32, tag="mm")
                    for n2 in range(2):
                        col = (c2 * 2 + n2) * 512
                        for kk in range(2):
                            nc.tensor.matmul(
                                ps[:, n2 * 512:(n2 + 1) * 512],
                                wk_sb[:, 2 * kk:2 * kk + 2, m, :],
                                pron[:, 2 * kk:2 * kk + 2, col:col + 512],
                                start=(kk == 0), stop=(kk == 1), perf_mode=DR)
                    nc.scalar.activation(k_sb[:, m, c2 * 1024:(c2 + 1) * 1024],
                                         ps[:], AF.Identity, scale=1.0 / WS,
                                         bias=bk_sb[:, m:m + 1])
            # V (transposed, kpos-major): out v_sb[:, mt, 0:1024]
            for mt in range(NT_K):
                ps = ps_mm.tile([P, 1024], F32, tag="mm")
                for n2 in range(2):
                    for kk in range(2):
                        nc.tensor.matmul(
                            ps[:, n2 * 512:(n2 + 1) * 512],
                            pron[:, 2 * kk:2 * kk + 2, mt * P:(mt + 1) * P],
                            wv_sb[:, 2 * kk:2 * kk + 2, n2 * 512:(n2 + 1) * 512],
                            start=(kk == 0), stop=(kk == 1), perf_mode=DR)
                nc.vector.tensor_scalar(v_sb[:, mt, :], ps[:], 1.0 / WS, None,
                                        ALU.mult)
        es_norm.close()
        es_qw.close()

        # FFN weights arrive during attention
        es_fw = ExitStack()
        pfw = es_fw.enter_context(tc.tile_pool(name="pfw", bufs=1, side="right"))
        w1q_sb = pfw.tile([P, M_SEM, M_FF, P], FP8)
        w2q_sb = pfw.tile([P, M_FF, M_SEM, P], FP8)
        for c4 in range(4):
            nc.sync.dma_start(w1q_sb[:, 2 * c4:2 * c4 + 2], w1q[:, 2 * c4:2 * c4 + 2])
        for c4 in range(4):
            nc.sync.dma_start(w2q_sb[:, 8 * c4:8 * c4 + 8], w2q[:, 8 * c4:8 * c4 + 8])
        wo_sb = pfw.tile([P, M_SEM, M_SEM, P], FP8)
        nc.sync.dma_start(wo_sb[:], wo[:])

        # ---------- attention ----------
        with tc.tile_pool(name="pattn", bufs=1) as pattn, \
             tc.tile_pool(name="ps_att", bufs=1, space="PSUM") as ps_att:
            for n in range(2):
                for h in range(H):
                    pt = pattn.tile([P, NT_K, 512], FP8, tag="pt", bufs=2)
                    den = ps_att.tile([P, 512], F32, tag="den", bufs=2)
                    cps = ps_att.tile([P, 512], F32, tag="pv", bufs=2)
                    for kp in range(8):
                        sc = ps_att.tile([P, 1024], F32, tag="sc", bufs=2)
                        nc.tensor.matmul(
                            sc[:, 0:512],
                            k_sb[:, h, (2 * kp) * P:(2 * kp + 1) * P],
                            q_sb[:, h, n * 512:(n + 1) * 512],
                            start=True, stop=True)
                        nc.tensor.matmul(
                            sc[:, 512:1024],
                            k_sb[:, h, (2 * kp + 1) * P:(2 * kp + 2) * P],
                            q_sb[:, h, n * 512:(n + 1) * 512],
                            start=True, stop=True)
                        nc.scalar.activation(
                            pt[:, 2 * kp:2 * kp + 2, :].rearrange(
                                "p a b -> p (a b)"),
                            sc[:], AF.Exp, scale=QK_SCALE)
                        nc.tensor.matmul(den[:], ones2_f8[:],
                                         pt[:, 2 * kp:2 * kp + 2, :],
                                         start=(kp == 0), stop=(kp == 7),
                                         perf_mode=DR)
                        nc.tensor.matmul(cps[:],
                                         v_sb[:, 2 * kp:2 * kp + 2,
                                              h * P:(h + 1) * P],
                                         pt[:, 2 * kp:2 * kp + 2, :],
                                         start=(kp == 0), stop=(kp == 7),
                                         perf_mode=DR)
                    bc_sb = pattn.tile([P, 512], F32, tag="bcsb", bufs=2)
                    nc.vector.reciprocal_approx_fast(bc_sb[:], den[:])
                    nc.vector.tensor_tensor(ctx_n[n][:, h, :], cps[:], bc_sb[:],
                                            op=ALU.mult)
        es_att.close()

        # ---------- out-proj + ffn-norm + FFN (per half) ----------
        with tc.tile_pool(name="pff", bufs=1) as pff, \
             tc.tile_pool(name="ps_ff", bufs=2, space="PSUM") as ps_ff:
            xq_n = [pff.tile([P, M_SEM, 512], FP8, name=f"xq{n}")
                    for n in range(2)]
            h2_n = [pff.tile([P, M_FF, 512], FP8, name=f"h2{n}")
                    for n in range(2)]

            def oproj(n):
                ncol = slice(n * 512, (n + 1) * 512)
                t_o = pff.tile([P, M_SEM, 512], BF16, tag="t_o", bufs=2)
                for m in range(M_SEM):
                    ps = ps_ff.tile([P, 512], F32, tag="mmf")
                    for kk in range(4):
                        nc.tensor.matmul(ps[:], wo_sb[:, 2 * kk:2 * kk + 2, m, :],
                                         ctx_n[n][:, 2 * kk:2 * kk + 2, :],
                                         start=(kk == 0), stop=(kk == 3),
                                         perf_mode=DR)
                    nc.vector.tensor_scalar(t_o[:, m, :], ps[:],
                                            1.0 / (WS * CTX_S),
                                            obias_sb[:, m:m + 1],
                                            ALU.mult, ALU.add)
                semo = semT_sb[:, :, ncol]
                nc.vector.tensor_tensor(semo, t_o[:], semo, op=ALU.add)

            def xnorm(n):
                ncol = slice(n * 512, (n + 1) * 512)
                semo = semT_sb[:, :, ncol]
                sqf = pff.tile([P, M_SEM, 512], BF16, tag="sqf", bufs=1)
                nc.scalar.activation(sqf[:], semo, AF.Square)
                pst = ps_ff.tile([P, 512], F32, tag="nrmf")
                for m in range(M_SEM):
                    nc.tensor.matmul(pst[:], ones_full[:], sqf[:, m, :],
                                     start=(m == 0), stop=(m == M_SEM - 1))
                rsbc = pff.tile([P, 512], F32, tag="rsbc", bufs=1)
                nc.scalar.activation(rsbc[:], pst[:], AF.Ln, bias=eps_t[:],
                                     scale=1.0 / DS)
                nc.scalar.activation(rsbc[:], rsbc[:], AF.Exp, scale=-0.5)
                for m in range(M_SEM):
                    nc.vector.scalar_tensor_tensor(
                        out=xq_n[n][:, m, :], in0=semT_sb[:, m, ncol],
                        scalar=gff_sb[:, m:m + 1], in1=rsbc[:],
                        op0=ALU.mult, op1=ALU.mult)

            def ffn1(n):
                for m in range(M_FF):
                    ps = ps_ff.tile([P, 512], F32, tag="mmf")
                    for kk in range(4):
                        nc.tensor.matmul(ps[:], w1q_sb[:, 2 * kk:2 * kk + 2, m, :],
                                         xq_n[n][:, 2 * kk:2 * kk + 2, :],
                                         start=(kk == 0), stop=(kk == 3),
                                         perf_mode=DR)
                    sn = pff.tile([P, 512], BF16, tag="sn", bufs=3)
                    nc.scalar.activation(sn[:], ps[:], AF.Sin, bias=hpi_t[:],
                                         scale=acos_sb[:, m:m + 1])
                    nc.vector.scalar_tensor_tensor(
                        out=h2_n[n][:, m, :], in0=sn[:],
                        scalar=nrb2_sb[:, m:m + 1], in1=ps[:],
                        op0=ALU.mult, op1=ALU.add)

            def ffn2(n):
                ncol = slice(n * 512, (n + 1) * 512)
                yf = pff.tile([P, M_SEM, 512], BF16, tag="yf", bufs=1)
                for m in range(M_SEM):
                    ps = ps_ff.tile([P, 512], F32, tag="mmf")
                    for kk in range(16):
                        nc.tensor.matmul(ps[:], w2q_sb[:, 2 * kk:2 * kk + 2, m, :],
                                         h2_n[n][:, 2 * kk:2 * kk + 2, :],
                                         start=(kk == 0), stop=(kk == 15),
                                         perf_mode=DR)
                    nc.vector.tensor_scalar(yf[:, m, :], ps[:], dqs_sb[:],
                                            ybias_sb[:, m:m + 1],
                                            ALU.mult, ALU.add)
                yo = pff.tile([P, M_SEM, 512], F32, tag="yo", bufs=1)
                nc.vector.tensor_tensor(yo[:], yf[:], semT_sb[:, :, ncol],
                                        op=ALU.add)
                for m2 in range(2):
                    nc.sync.dma_start(
                        outT_r[:, 4 * m2:4 * m2 + 4, ncol],
                        yo[:, 4 * m2:4 * m2 + 4, :])

            oproj(0)
            oproj(1)
            xnorm(0)
            ffn1(0)
            xnorm(1)
            ffn2(0)
            ffn1(1)
            ffn2(1)
        es_fw.close()

    nc.compile()
    return nc


_NC_CACHE = {}


def _get_nc(debug_outs=False):
    key = bool(debug_outs)
    if key not in _NC_CACHE:
        _NC_CACHE[key] = build_nc(debug_outs)
    return _NC_CACHE[key]


def make_in_maps(inputs):
    """Host-side shard + layout prep. inputs: dict of full np arrays."""
    import ml_dtypes
    f8 = ml_dtypes.float8_e4m3
    f32 = np.float32
    sem = np.asarray(inputs["sem"], f32)
    pro = np.asarray(inputs["pro"], f32)

    def cols(v, nm):
        return np.ascontiguousarray(np.asarray(v, f32).reshape(nm, P).T)

    def wlay(wT, nkt, nm):
        # [in=nkt*128, out=nm*128] -> [128p, nkt, nm, 128c]
        return np.ascontiguousarray(
            wT.reshape(nkt, P, nm, P).transpose(1, 0, 2, 3))

    WqT = np.asarray(inputs["Wq"], f32).T * WS
    WkT = np.asarray(inputs["Wk"], f32).T * WS
    WvT = np.asarray(inputs["Wv"], f32).T * WS
    WoT = np.asarray(inputs["Wo"], f32).T * WS

    W1 = np.asarray(inputs["W1"], f32)
    W2 = np.asarray(inputs["W2"], f32)
    mw1 = np.maximum(np.abs(W1).mean(), 1e-5)
    mw2 = np.maximum(np.abs(W2).mean(), 1e-5)
    w1s = np.clip(np.round(W1 / mw1), -1, 1)          # [DF, DS]
    w2s = np.clip(np.round(W2 / mw2), -1, 1)          # [DS, DF]

    alpha = np.asarray(inputs["alpha"], f32)
    beta = np.asarray(inputs["beta"], f32)
    acos_v = 2.0 * alpha * mw1
    rb2 = 1.0 / (2.0 * (beta + 1e-9) * mw1)           # positive snake const
    ybias_v = (mw1 * mw2) * (w2s @ rb2)               # [DS]
    obias_v = np.asarray(inputs["bo"], f32) + \
        np.asarray(inputs["Wo"], f32) @ np.asarray(inputs["bv"], f32)

    common = {
        "gsem": cols(inputs["g_sem"], M_SEM),
        "gpro": cols(inputs["g_pro"], M_PRO),
        "gff": cols(inputs["g_ff"], M_SEM),
        "bq": cols(inputs["bq"], M_SEM),
        "bk": cols(inputs["bk"], M_SEM),
        "obias": cols(obias_v, M_SEM),
        "ybias": cols(ybias_v, M_SEM),
        "acos": cols(acos_v, M_FF),
        "nrb2": cols(-rb2, M_FF),
        "dqs": np.full((P, 1), mw1 * mw2, f32),
        "wq": wlay(WqT, M_SEM, M_SEM).astype(f8),
        "wk": wlay(WkT, M_PRO, M_SEM).astype(f8),
        "wv": np.ascontiguousarray(
            WvT.reshape(M_PRO, P, DS).transpose(1, 0, 2)).astype(f8),
        "wo": wlay(WoT, M_SEM, M_SEM).astype(f8),
        "w1q": wlay(np.ascontiguousarray(w1s.T), M_SEM, M_FF).astype(f8),
        "w2q": wlay(np.ascontiguousarray(w2s.T), M_FF, M_SEM).astype(f8),
    }

    in_maps = []
    for c in range(N_CORES):
        b, half = c // 2, c % 2
        m = dict(common)
        m["semT"] = np.ascontiguousarray(sem[b, half * TOK:(half + 1) * TOK, :].T)
        m["proT"] = np.ascontiguousarray(pro[b].T).astype(ml_dtypes.bfloat16)
        in_maps.append(m)
    return in_maps


def assemble_out(results):
    out = np.empty((B, S, DS), np.float32)
    for c in range(N_CORES):
        b, half = c // 2, c % 2
        out[b, half * TOK:(half + 1) * TOK, :] = results[c]["outT"].T
    return out


def kernel(**inputs):
    nc = _get_nc()
    in_maps = make_in_maps(inputs)
    res = run_bass_kernel_spmd(nc, in_maps, core_ids=list(range(N_CORES)))
    return assemble_out(res.results)


# revision 34
# speedup vs baseline: 1.3349x; 1.0274x over previous
"""Trainium2 Bass kernel for nn_CrossAttentionFusion (cross-attention + BitLinear FFN).

Sharding: 8 cores = 4 batches x 2 sequence-halves (data parallel, no collectives).
Each core owns 1024 query tokens; computes K/V for its batch's full 2048 tokens.

v2: fp8 DoubleRow matmuls everywhere (QKV/O/PV/FFN), host-side ternarization of
W1/W2 (shipped as fp8 +-1/0), softmax denominator computed on the PE via a
ones-matmul, act_quant realized as a direct fp8 cast.
"""
import math
import numpy as np
from contextlib import ExitStack

import concourse.bass as bass
import concourse.bass_isa as bass_isa
import concourse.tile as tile
from concourse import bacc, mybir
from concourse.bass_utils import run_bass_kernel_spmd

F32 = mybir.dt.float32
F32R = mybir.dt.float32r
BF16 = mybir.dt.bfloat16
FP8 = mybir.dt.float8e4
AF = mybir.ActivationFunctionType
ALU = mybir.AluOpType
DR = mybir.MatmulPerfMode.DoubleRow

B, S, DS, DP, H = 4, 2048, 1024, 512, 8
DF = 4 * DS
HD = DS // H          # 128
TOK = 1024            # query tokens per core
N_CORES = 8
EPS = 1e-6
QK_SCALE = 1.0 / math.sqrt(HD)
HALF_PI = math.pi / 2.0
WS = 32.0             # fp8 scale for Wq/Wk/Wv/Wo
CTX_S = 64.0          # fp8 scale for ctx (folded into 1/den via ones=1/64)

P = 128
M_SEM = DS // P       # 8
M_PRO = DP // P       # 4
M_FF = DF // P        # 32
NT_K = S // P         # 16


def build_nc(debug_outs=False):
    nc = bacc.Bacc("TRN2", target_bir_lowering=False, debug=False, num_devices=N_CORES)

    semT = nc.dram_tensor("semT", [DS, TOK], F32, kind="ExternalInput").ap()
    proT = nc.dram_tensor("proT", [DP, S], BF16, kind="ExternalInput").ap()
    wq = nc.dram_tensor("wq", [P, M_SEM, M_SEM, P], FP8, kind="ExternalInput").ap()
    wk = nc.dram_tensor("wk", [P, M_PRO, M_SEM, P], FP8, kind="ExternalInput").ap()
    wv = nc.dram_tensor("wv", [P, M_PRO, DS], FP8, kind="ExternalInput").ap()
    wo = nc.dram_tensor("wo", [P, M_SEM, M_SEM, P], FP8, kind="ExternalInput").ap()
    w1q = nc.dram_tensor("w1q", [P, M_SEM, M_FF, P], FP8, kind="ExternalInput").ap()
    w2q = nc.dram_tensor("w2q", [P, M_FF, M_SEM, P], FP8, kind="ExternalInput").ap()
    gsem = nc.dram_tensor("gsem", [P, M_SEM], F32, kind="ExternalInput").ap()
    gpro = nc.dram_tensor("gpro", [P, M_PRO], F32, kind="ExternalInput").ap()
    gff = nc.dram_tensor("gff", [P, M_SEM], F32, kind="ExternalInput").ap()
    bq = nc.dram_tensor("bq", [P, M_SEM], F32, kind="ExternalInput").ap()
    bk = nc.dram_tensor("bk", [P, M_SEM], F32, kind="ExternalInput").ap()
    obias = nc.dram_tensor("obias", [P, M_SEM], F32, kind="ExternalInput").ap()
    ybias = nc.dram_tensor("ybias", [P, M_SEM], F32, kind="ExternalInput").ap()
    acos = nc.dram_tensor("acos", [P, M_FF], F32, kind="ExternalInput").ap()
    nrb2 = nc.dram_tensor("nrb2", [P, M_FF], F32, kind="ExternalInput").ap()
    dqs = nc.dram_tensor("dqs", [P, 1], F32, kind="ExternalInput").ap()
    outT = nc.dram_tensor("outT", [DS, TOK], F32, kind="ExternalOutput").ap()

    semT_r = semT.rearrange("(m p) t -> p m t", p=P)
    proT_r = proT.rearrange("(m p) t -> p m t", p=P)
    outT_r = outT.rearrange("(m p) t -> p m t", p=P)

    with tile.TileContext(nc) as tc, ExitStack() as top:
        persist = top.enter_context(tc.tile_pool(name="persist", bufs=1))

        ones = persist.tile([P, 1], BF16)
        nc.vector.memset(ones[:], 1.0)
        ones_full = persist.tile([P, P], BF16)
        nc.vector.memset(ones_full[:], 1.0)
        ones_row = persist.tile([1, P], BF16)
        nc.vector.memset(ones_row[:], 1.0)
        ones2_f8 = persist.tile([P, 2, P], FP8)
        nc.vector.memset(ones2_f8[:], 1.0 / CTX_S)
        eps_t = persist.tile([P, 1], F32)
        nc.vector.memset(eps_t[:], EPS)
        hpi_t = persist.tile([P, 1], F32)
        nc.vector.memset(hpi_t[:], HALF_PI)

        gsem_sb = persist.tile([P, M_SEM], F32)
        gpro_sb = persist.tile([P, M_PRO], F32)
        gff_sb = persist.tile([P, M_SEM], F32)
        bq_sb = persist.tile([P, M_SEM], F32)
        bk_sb = persist.tile([P, M_SEM], F32)
        obias_sb = persist.tile([P, M_SEM], F32)
        ybias_sb = persist.tile([P, M_SEM], F32)
        acos_sb = persist.tile([P, M_FF], F32)
        nrb2_sb = persist.tile([P, M_FF], F32)
        dqs_sb = persist.tile([P, 1], F32)
        for ap_d, t in [(gsem, gsem_sb), (gpro, gpro_sb), (gff, gff_sb),
                        (bq, bq_sb), (bk, bk_sb), (obias, obias_sb),
                        (ybias, ybias_sb), (acos, acos_sb), (nrb2, nrb2_sb),
                        (dqs, dqs_sb)]:
            nc.sync.dma_start(t[:], ap_d[:])

        # persistent big tensors
        semT_sb = persist.tile([P, M_SEM, TOK], F32)      # becomes semout in place
        es_att = ExitStack()
        patt_io = es_att.enter_context(tc.tile_pool(name="patt_io", bufs=1))
        q_sb = patt_io.tile([P, M_SEM, TOK], FP8)
        k_sb = patt_io.tile([P, M_SEM, S], FP8)
        v_sb = patt_io.tile([P, NT_K, DS], FP8)
        ctx_n = [persist.tile([P, M_SEM, 512], FP8, name=f"ctx{n}")
                 for n in range(2)]

        for m in range(M_SEM):
            nc.sync.dma_start(semT_sb[:, m, :], semT_r[:, m, :])

        # ---------- input norms -> semn/pron in fp8 ----------
        es_norm = ExitStack()
        pnorm = es_norm.enter_context(tc.tile_pool(name="pnorm", bufs=1))
        semn = pnorm.tile([P, M_SEM, TOK], FP8)
        pron = pnorm.tile([P, M_PRO, S], FP8)
        ps_nrm = es_norm.enter_context(
            tc.tile_pool(name="ps_nrm", bufs=2, space="PSUM"))

        with tc.tile_pool(name="pnsc", bufs=1) as pnsc:
            proT_sb = pnsc.tile([P, M_PRO, S], BF16)
            for m in range(M_PRO):
                nc.sync.dma_start(proT_sb[:, m, :], proT_r[:, m, :])

            def rmsnorm(x_sb, nm, T, g, out_f8):
                D = nm * P
                sq = pnsc.tile([P, nm * T], BF16, tag="sq", bufs=1, name="sq")
                for m in range(nm):
                    nc.scalar.activation(sq[:, m * T:(m + 1) * T], x_sb[:, m, :],
                                         AF.Square)
                rs_bc = pnsc.tile([P, T], F32, tag=f"rsbc{nm}", bufs=1)
                for ch in range(T // 512):
                    pst = ps_nrm.tile([P, 512], F32, tag="nrm")
                    for m in range(nm):
                        nc.tensor.matmul(
                            pst[:], ones_full[:],
                            sq[:, m * T + ch * 512:m * T + (ch + 1) * 512],
                            start=(m == 0), stop=(m == nm - 1))
                    nc.scalar.activation(rs_bc[:, ch * 512:(ch + 1) * 512],
                                         pst[:], AF.Ln, bias=eps_t[:],
                                         scale=1.0 / D)
                nc.scalar.activation(rs_bc[:], rs_bc[:], AF.Exp, scale=-0.5)
                for m in range(nm):
                    nc.vector.scalar_tensor_tensor(
                        out=out_f8[:, m, :], in0=x_sb[:, m, :],
                        scalar=g[:, m:m + 1], in1=rs_bc[:],
                        op0=ALU.mult, op1=ALU.mult)

            rmsnorm(semT_sb, M_SEM, TOK, gsem_sb, semn)
            rmsnorm(proT_sb, M_PRO, S, gpro_sb, pron)

        # ---------- QKV projections (fp8 DoubleRow) ----------
        es_qw = ExitStack()
        pqw = es_qw.enter_context(tc.tile_pool(name="pqw", bufs=1, side="right"))
        wq_sb = pqw.tile([P, M_SEM, M_SEM, P], FP8)
        wk_sb = pqw.tile([P, M_PRO, M_SEM, P], FP8)
        wv_sb = pqw.tile([P, M_PRO, DS], FP8)
        nc.sync.dma_start(wq_sb[:], wq[:])
        nc.sync.dma_start(wk_sb[:], wk[:])
        nc.sync.dma_start(wv_sb[:], wv[:])

        with tc.tile_pool(name="ps_mm", bufs=2, space="PSUM") as ps_mm:
            # Q: out q_sb[:, m, 0:1024]
            for m in range(M_SEM):
                ps = ps_mm.tile([P, 1024], F32, tag="mm")
                for n2 in range(2):
                    for kk in range(4):
                        nc.tensor.matmul(
                            ps[:, n2 * 512:(n2 + 1) * 512],
                            wq_sb[:, 2 * kk:2 * kk + 2, m, :],
                            semn[:, 2 * kk:2 * kk + 2, n2 * 512:(n2 + 1) * 512],
                            start=(kk == 0), stop=(kk == 3), perf_mode=DR)
                nc.vector.tensor_scalar(q_sb[:, m, :], ps[:], 1.0 / WS,
                                        bq_sb[:, m:m + 1], ALU.mult, ALU.add)
            # K: out k_sb[:, m, 0:2048]
            for m in range(M_SEM):
                for c2 in range(2):
                    ps = ps_mm.tile([P, 1024], F32, tag="mm")
                    for n2 in range(2):
                        col = (c2 * 2 + n2) * 512
                        for kk in range(2):
                            nc.tensor.matmul(
                                ps[:, n2 * 512:(n2 + 1) * 512],
                                wk_sb[:, 2 * kk:2 * kk + 2, m, :],
                                pron[:, 2 * kk:2 * kk + 2, col:col + 512],
                                start=(kk == 0), stop=(kk == 1), perf_mode=DR)
                    nc.scalar.activation(k_sb[:, m, c2 * 1024:(c2 + 1) * 1024],
                                         ps[:], AF.Identity, scale=1.0 / WS,
                                         bias=bk_sb[:, m:m + 1])
            # V (transposed, kpos-major): out v_sb[:, mt, 0:1024]
            for mt in range(NT_K):
                ps = ps_mm.tile([P, 1024], F32, tag="mm")
                for n2 in range(2):
                    for kk in range(2):
                        nc.tensor.matmul(
                            ps[:, n2 * 512:(n2 + 1) * 512],
                            pron[:, 2 * kk:2 * kk + 2, mt * P:(mt + 1) * P],
                            wv_sb[:, 2 * kk:2 * kk + 2, n2 * 512:(n2 + 1) * 512],
                            start=(kk == 0), stop=(kk == 1), perf_mode=DR)
                nc.vector.tensor_scalar(v_sb[:, mt, :], ps[:], 1.0 / WS, None,
                                        ALU.mult)
        es_norm.close()
        es_qw.close()

        # FFN weights arrive during attention
        es_fw = ExitStack()
        pfw = es_fw.enter_context(tc.tile_pool(name="pfw", bufs=1, side="right"))
        w1q_sb = pfw.tile([P, M_SEM, M_FF, P], FP8)
        w2q_sb = pfw.tile([P, M_FF, M_SEM, P], FP8)
        for c4 in range(4):
            nc.sync.dma_start(w1q_sb[:, 2 * c4:2 * c4 + 2], w1q[:, 2 * c4:2 * c4 + 2])
        for c4 in range(4):
            nc.sync.dma_start(w2q_sb[:, 8 * c4:8 * c4 + 8], w2q[:, 8 * c4:8 * c4 + 8])
        wo_sb = pfw.tile([P, M_SEM, M_SEM, P], FP8)
        nc.sync.dma_start(wo_sb[:], wo[:])

        # ---------- attention ----------
        with tc.tile_pool(name="pattn", bufs=1) as pattn, \
             tc.tile_pool(name="ps_att", bufs=1, space="PSUM") as ps_att:
            for n in range(2):
                for h in range(H):
                    pt = pattn.tile([P, NT_K, 512], FP8, tag="pt", bufs=2)
                    den = ps_att.tile([P, 512], F32, tag="den", bufs=2)
                    cps = ps_att.tile([P, 512], F32, tag="pv", bufs=2)
                    for kp in range(8):
                        sc = ps_att.tile([P, 1024], F32, tag="sc", bufs=2)
                        nc.tensor.matmul(
                            sc[:, 0:512],
                            k_sb[:, h, (2 * kp) * P:(2 * kp + 1) * P],
                            q_sb[:, h, n * 512:(n + 1) * 512],
                            start=True, stop=True)
                        nc.tensor.matmul(
                            sc[:, 512:1024],
                            k_sb[:, h, (2 * kp + 1) * P:(2 * kp + 2) * P],
                            q_sb[:, h, n * 512:(n + 1) * 512],
                            start=True, stop=True)
                        nc.scalar.activation(
                            pt[:, 2 * kp:2 * kp + 2, :].rearrange(
                                "p a b -> p (a b)"),
                            sc[:], AF.Exp, scale=QK_SCALE)
                        nc.tensor.matmul(den[:], ones2_f8[:],
                                         pt[:, 2 * kp:2 * kp + 2, :],
                                         start=(kp == 0), stop=(kp == 7),
                                         perf_mode=DR)
                        nc.tensor.matmul(cps[:],
                                         v_sb[:, 2 * kp:2 * kp + 2,
                                              h * P:(h + 1) * P],
                                         pt[:, 2 * kp:2 * kp + 2, :],
                                         start=(kp == 0), stop=(kp == 7),
                                         perf_mode=DR)
                    bc_sb = pattn.tile([P, 512], F32, tag="bcsb", bufs=2)
                    nc.vector.reciprocal_approx_fast(bc_sb[:], den[:])
                    nc.vector.tensor_tensor(ctx_n[n][:, h, :], cps[:], bc_sb[:],
                                            op=ALU.mult)
        es_att.close()

        # ---------- out-proj + ffn-norm + FFN (per half) ----------
        with tc.tile_pool(name="pff", bufs=1) as pff, \
             tc.tile_pool(name="ps_ff", bufs=2, space="PSUM") as ps_ff:
            xq_n = [pff.tile([P, M_SEM, 512], FP8, name=f"xq{n}")
                    for n in range(2)]
            h2_n = [pff.tile([P, M_FF, 512], FP8, name=f"h2{n}")
                    for n in range(2)]

            def oproj(n):
                ncol = slice(n * 512, (n + 1) * 512)
                t_o = pff.tile([P, M_SEM, 512], BF16, tag="t_o", bufs=2)
                for m in range(M_SEM):
                    ps = ps_ff.tile([P, 512], F32, tag="mmf")
                    for kk in range(4):
                        nc.tensor.matmul(ps[:], wo_sb[:, 2 * kk:2 * kk + 2, m, :],
                                         ctx_n[n][:, 2 * kk:2 * kk + 2, :],
                                         start=(kk == 0), stop=(kk == 3),
                                         perf_mode=DR)
                    nc.vector.tensor_scalar(t_o[:, m, :], ps[:],
                                            1.0 / (WS * CTX_S),
                                            obias_sb[:, m:m + 1],
                                            ALU.mult, ALU.add)
                semo = semT_sb[:, :, ncol]
                nc.vector.tensor_tensor(semo, t_o[:], semo, op=ALU.add)

            def xnorm(n):
                ncol = slice(n * 512, (n + 1) * 512)
                semo = semT_sb[:, :, ncol]
                sqf = pff.tile([P, M_SEM, 512], BF16, tag="sqf", bufs=1)
                nc.scalar.activation(sqf[:], semo, AF.Square)
                pst = ps_ff.tile([P, 512], F32, tag="nrmf")
                for m in range(M_SEM):
                    nc.tensor.matmul(pst[:], ones_full[:], sqf[:, m, :],
                                     start=(m == 0), stop=(m == M_SEM - 1))
                rsbc = pff.tile([P, 512], F32, tag="rsbc", bufs=1)
                nc.scalar.activation(rsbc[:], pst[:], AF.Ln, bias=eps_t[:],
                                     scale=1.0 / DS)
                nc.scalar.activation(rsbc[:], rsbc[:], AF.Exp, scale=-0.5)
                for m in range(M_SEM):
                    nc.vector.scalar_tensor_tensor(
                        out=xq_n[n][:, m, :], in0=semT_sb[:, m, ncol],
                        scalar=gff_sb[:, m:m + 1], in1=rsbc[:],
                        op0=ALU.mult, op1=ALU.mult)

            def ffn1(n):
                for m in range(M_FF):
                    ps = ps_ff.tile([P, 512], F32, tag="mmf")
                    for kk in range(4):
                        nc.tensor.matmul(ps[:], w1q_sb[:, 2 * kk:2 * kk + 2, m, :],
                                         xq_n[n][:, 2 * kk:2 * kk + 2, :],
                                         start=(kk == 0), stop=(kk == 3),
                                         perf_mode=DR)
                    sn = pff.tile([P, 512], BF16, tag="sn", bufs=3)
                    nc.scalar.activation(sn[:], ps[:], AF.Sin, bias=hpi_t[:],
                                         scale=acos_sb[:, m:m + 1])
                    nc.vector.scalar_tensor_tensor(
                        out=h2_n[n][:, m, :], in0=sn[:],
                        scalar=nrb2_sb[:, m:m + 1], in1=ps[:],
                        op0=ALU.mult, op1=ALU.add)

            def ffn2(n):
                ncol = slice(n * 512, (n + 1) * 512)
                yf = pff.tile([P, M_SEM, 512], BF16, tag="yf", bufs=1)
                for m in range(M_SEM):
                    ps = ps_ff.tile([P, 512], F32, tag="mmf")
                    for kk in range(16):
                        nc.tensor.matmul(ps[:], w2q_sb[:, 2 * kk:2 * kk + 2, m, :],
                                         h2_n[n][:, 2 * kk:2 * kk + 2, :],
                                         start=(kk == 0), stop=(kk == 15),
                                         perf_mode=DR)
                    nc.vector.tensor_scalar(yf[:, m, :], ps[:], dqs_sb[:],
                                            ybias_sb[:, m:m + 1],
                                            ALU.mult, ALU.add)
                yo = pff.tile([P, M_SEM, 512], F32, tag="yo", bufs=1)
                for m2 in range(4):
                    nc.vector.tensor_tensor(
                        yo[:, 2 * m2:2 * m2 + 2, :],
                        yf[:, 2 * m2:2 * m2 + 2, :],
                        semT_sb[:, 2 * m2:2 * m2 + 2, ncol], op=ALU.add)
                    nc.sync.dma_start(
                        outT_r[:, 2 * m2:2 * m2 + 2, ncol],
                        yo[:, 2 * m2:2 * m2 + 2, :])

            oproj(0)
            oproj(1)
            xnorm(0)
            ffn1(0)
            xnorm(1)
            ffn2(0)
            ffn1(1)
            ffn2(1)
        es_fw.close()

    nc.compile()
    return nc


_NC_CACHE = {}


def _get_nc(debug_outs=False):
    key = bool(debug_outs)
    if key not in _NC_CACHE:
        _NC_CACHE[key] = build_nc(debug_outs)
    return _NC_CACHE[key]


def make_in_maps(inputs):
    """Host-side shard + layout prep. inputs: dict of full np arrays."""
    import ml_dtypes
    f8 = ml_dtypes.float8_e4m3
    f32 = np.float32
    sem = np.asarray(inputs["sem"], f32)
    pro = np.asarray(inputs["pro"], f32)

    def cols(v, nm):
        return np.ascontiguousarray(np.asarray(v, f32).reshape(nm, P).T)

    def wlay(wT, nkt, nm):
        # [in=nkt*128, out=nm*128] -> [128p, nkt, nm, 128c]
        return np.ascontiguousarray(
            wT.reshape(nkt, P, nm, P).transpose(1, 0, 2, 3))

    WqT = np.asarray(inputs["Wq"], f32).T * WS
    WkT = np.asarray(inputs["Wk"], f32).T * WS
    WvT = np.asarray(inputs["Wv"], f32).T * WS
    WoT = np.asarray(inputs["Wo"], f32).T * WS

    W1 = np.asarray(inputs["W1"], f32)
    W2 = np.asarray(inputs["W2"], f32)
    mw1 = np.maximum(np.abs(W1).mean(), 1e-5)
    mw2 = np.maximum(np.abs(W2).mean(), 1e-5)
    w1s = np.clip(np.round(W1 / mw1), -1, 1)          # [DF, DS]
    w2s = np.clip(np.round(W2 / mw2), -1, 1)          # [DS, DF]

    alpha = np.asarray(inputs["alpha"], f32)
    beta = np.asarray(inputs["beta"], f32)
    acos_v = 2.0 * alpha * mw1
    rb2 = 1.0 / (2.0 * (beta + 1e-9) * mw1)           # positive snake const
    ybias_v = (mw1 * mw2) * (w2s @ rb2)               # [DS]
    obias_v = np.asarray(inputs["bo"], f32) + \
        np.asarray(inputs["Wo"], f32) @ np.asarray(inputs["bv"], f32)

    common = {
        "gsem": cols(inputs["g_sem"], M_SEM),
        "gpro": cols(inputs["g_pro"], M_PRO),
        "gff": cols(inputs["g_ff"], M_SEM),
        "bq": cols(inputs["bq"], M_SEM),
        "bk": cols(inputs["bk"], M_SEM),
        "obias": cols(obias_v, M_SEM),
        "ybias": cols(ybias_v, M_SEM),
        "acos": cols(acos_v, M_FF),
        "nrb2": cols(-rb2, M_FF),
        "dqs": np.full((P, 1), mw1 * mw2, f32),
        "wq": wlay(WqT, M_SEM, M_SEM).astype(f8),
        "wk": wlay(WkT, M_PRO, M_SEM).astype(f8),
        "wv": np.ascontiguousarray(
            WvT.reshape(M_PRO, P, DS).transpose(1, 0, 2)).astype(f8),
        "wo": wlay(WoT, M_SEM, M_SEM).astype(f8),
        "w1q": wlay(np.ascontiguousarray(w1s.T), M_SEM, M_FF).astype(f8),
        "w2q": wlay(np.ascontiguousarray(w2s.T), M_FF, M_SEM).astype(f8),
    }

    in_maps = []
    for c in range(N_CORES):
        b, half = c // 2, c % 2
        m = dict(common)
        m["semT"] = np.ascontiguousarray(sem[b, half * TOK:(half + 1) * TOK, :].T)
        m["proT"] = np.ascontiguousarray(pro[b].T).astype(ml_dtypes.bfloat16)
        in_maps.append(m)
    return in_maps


def assemble_out(results):
    out = np.empty((B, S, DS), np.float32)
    for c in range(N_CORES):
        b, half = c // 2, c % 2
        out[b, half * TOK:(half + 1) * TOK, :] = results[c]["outT"].T
    return out


def kernel(**inputs):
    nc = _get_nc()
    in_maps = make_in_maps(inputs)
    res = run_bass_kernel_spmd(nc, in_maps, core_ids=list(range(N_CORES)))
    return assemble_out(res.results)


# revision 36
# speedup vs baseline: 1.3362x; 1.0010x over previous
"""Trainium2 Bass kernel for nn_CrossAttentionFusion (cross-attention + BitLinear FFN).

Sharding: 8 cores = 4 batches x 2 sequence-halves (data parallel, no collectives).
Each core owns 1024 query tokens; computes K/V for its batch's full 2048 tokens.

v2: fp8 DoubleRow matmuls everywhere (QKV/O/PV/FFN), host-side ternarization of
W1/W2 (shipped as fp8 +-1/0), softmax denominator computed on the PE via a
ones-matmul, act_quant realized as a direct fp8 cast.
"""
import math
import numpy as np
from contextlib import ExitStack

import concourse.bass as bass
import concourse.bass_isa as bass_isa
import concourse.tile as tile
from concourse import bacc, mybir
from concourse.bass_utils import run_bass_kernel_spmd

F32 = mybir.dt.float32
F32R = mybir.dt.float32r
BF16 = mybir.dt.bfloat16
FP8 = mybir.dt.float8e4
AF = mybir.ActivationFunctionType
ALU = mybir.AluOpType
DR = mybir.MatmulPerfMode.DoubleRow

B, S, DS, DP, H = 4, 2048, 1024, 512, 8
DF = 4 * DS
HD = DS // H          # 128
TOK = 1024            # query tokens per core
N_CORES = 8
EPS = 1e-6
QK_SCALE = 1.0 / math.sqrt(HD)
HALF_PI = math.pi / 2.0
WS = 32.0             # fp8 scale for Wq/Wk/Wv/Wo
CTX_S = 64.0          # fp8 scale for ctx (folded into 1/den via ones=1/64)

P = 128
M_SEM = DS // P       # 8
M_PRO = DP // P       # 4
M_FF = DF // P        # 32
NT_K = S // P         # 16


def build_nc(debug_outs=False):
    nc = bacc.Bacc("TRN2", target_bir_lowering=False, debug=False, num_devices=N_CORES)

    semT = nc.dram_tensor("semT", [DS, TOK], F32, kind="ExternalInput").ap()
    proT = nc.dram_tensor("proT", [DP, S], BF16, kind="ExternalInput").ap()
    wq = nc.dram_tensor("wq", [P, M_SEM, M_SEM, P], FP8, kind="ExternalInput").ap()
    wk = nc.dram_tensor("wk", [P, M_PRO, M_SEM, P], FP8, kind="ExternalInput").ap()
    wv = nc.dram_tensor("wv", [P, M_PRO, DS], FP8, kind="ExternalInput").ap()
    wo = nc.dram_tensor("wo", [P, M_SEM, M_SEM, P], FP8, kind="ExternalInput").ap()
    w1q = nc.dram_tensor("w1q", [P, M_SEM, M_FF, P], FP8, kind="ExternalInput").ap()
    w2q = nc.dram_tensor("w2q", [P, M_FF, M_SEM, P], FP8, kind="ExternalInput").ap()
    gsem = nc.dram_tensor("gsem", [P, M_SEM], F32, kind="ExternalInput").ap()
    gpro = nc.dram_tensor("gpro", [P, M_PRO], F32, kind="ExternalInput").ap()
    gff = nc.dram_tensor("gff", [P, M_SEM], F32, kind="ExternalInput").ap()
    bq = nc.dram_tensor("bq", [P, M_SEM], F32, kind="ExternalInput").ap()
    bk = nc.dram_tensor("bk", [P, M_SEM], F32, kind="ExternalInput").ap()
    obias = nc.dram_tensor("obias", [P, M_SEM], F32, kind="ExternalInput").ap()
    ybias = nc.dram_tensor("ybias", [P, M_SEM], F32, kind="ExternalInput").ap()
    acos = nc.dram_tensor("acos", [P, M_FF], F32, kind="ExternalInput").ap()
    nrb2 = nc.dram_tensor("nrb2", [P, M_FF], F32, kind="ExternalInput").ap()
    dqs = nc.dram_tensor("dqs", [P, 1], F32, kind="ExternalInput").ap()
    outT = nc.dram_tensor("outT", [DS, TOK], F32, kind="ExternalOutput").ap()

    semT_r = semT.rearrange("(m p) t -> p m t", p=P)
    proT_r = proT.rearrange("(m p) t -> p m t", p=P)
    outT_r = outT.rearrange("(m p) t -> p m t", p=P)

    with tile.TileContext(nc) as tc, ExitStack() as top:
        persist = top.enter_context(tc.tile_pool(name="persist", bufs=1))

        ones = persist.tile([P, 1], BF16)
        nc.vector.memset(ones[:], 1.0)
        ones_full = persist.tile([P, P], BF16)
        nc.vector.memset(ones_full[:], 1.0)
        ones_row = persist.tile([1, P], BF16)
        nc.vector.memset(ones_row[:], 1.0)
        ones2_f8 = persist.tile([P, 2, P], FP8)
        nc.vector.memset(ones2_f8[:], 1.0 / CTX_S)
        eps_t = persist.tile([P, 1], F32)
        nc.vector.memset(eps_t[:], EPS)
        hpi_t = persist.tile([P, 1], F32)
        nc.vector.memset(hpi_t[:], HALF_PI)

        gsem_sb = persist.tile([P, M_SEM], F32)
        gpro_sb = persist.tile([P, M_PRO], F32)
        gff_sb = persist.tile([P, M_SEM], F32)
        bq_sb = persist.tile([P, M_SEM], F32)
        bk_sb = persist.tile([P, M_SEM], F32)
        obias_sb = persist.tile([P, M_SEM], F32)
        ybias_sb = persist.tile([P, M_SEM], F32)
        acos_sb = persist.tile([P, M_FF], F32)
        nrb2_sb = persist.tile([P, M_FF], F32)
        dqs_sb = persist.tile([P, 1], F32)
        for ap_d, t in [(gsem, gsem_sb), (gpro, gpro_sb), (gff, gff_sb),
                        (bq, bq_sb), (bk, bk_sb), (obias, obias_sb),
                        (ybias, ybias_sb), (acos, acos_sb), (nrb2, nrb2_sb),
                        (dqs, dqs_sb)]:
            nc.sync.dma_start(t[:], ap_d[:])

        # persistent big tensors
        semT_sb = persist.tile([P, M_SEM, TOK], F32)      # becomes semout in place
        es_att = ExitStack()
        patt_io = es_att.enter_context(tc.tile_pool(name="patt_io", bufs=1))
        q_sb = patt_io.tile([P, M_SEM, TOK], FP8)
        k_sb = patt_io.tile([P, M_SEM, S], FP8)
        v_sb = patt_io.tile([P, NT_K, DS], FP8)
        ctx_n = [persist.tile([P, M_SEM, 512], FP8, name=f"ctx{n}")
                 for n in range(2)]

        for m in range(M_SEM):
            nc.sync.dma_start(semT_sb[:, m, :], semT_r[:, m, :])

        # ---------- input norms -> semn/pron in fp8 ----------
        es_norm = ExitStack()
        pnorm = es_norm.enter_context(tc.tile_pool(name="pnorm", bufs=1))
        semn = pnorm.tile([P, M_SEM, TOK], FP8)
        pron = pnorm.tile([P, M_PRO, S], FP8)
        ps_nrm = es_norm.enter_context(
            tc.tile_pool(name="ps_nrm", bufs=2, space="PSUM"))

        with tc.tile_pool(name="pnsc", bufs=1) as pnsc:
            proT_sb = pnsc.tile([P, M_PRO, S], BF16)
            for m in range(M_PRO):
                nc.sync.dma_start(proT_sb[:, m, :], proT_r[:, m, :])

            def rmsnorm(x_sb, nm, T, g, out_f8):
                D = nm * P
                sq = pnsc.tile([P, nm * T], BF16, tag="sq", bufs=1, name="sq")
                for m in range(nm):
                    nc.scalar.activation(sq[:, m * T:(m + 1) * T], x_sb[:, m, :],
                                         AF.Square)
                rs_bc = pnsc.tile([P, T], F32, tag=f"rsbc{nm}", bufs=1)
                for ch in range(T // 512):
                    pst = ps_nrm.tile([P, 512], F32, tag="nrm")
                    for m in range(nm):
                        nc.tensor.matmul(
                            pst[:], ones_full[:],
                            sq[:, m * T + ch * 512:m * T + (ch + 1) * 512],
                            start=(m == 0), stop=(m == nm - 1))
                    nc.scalar.activation(rs_bc[:, ch * 512:(ch + 1) * 512],
                                         pst[:], AF.Ln, bias=eps_t[:],
                                         scale=1.0 / D)
                nc.scalar.activation(rs_bc[:], rs_bc[:], AF.Exp, scale=-0.5)
                for m in range(nm):
                    nc.vector.scalar_tensor_tensor(
                        out=out_f8[:, m, :], in0=x_sb[:, m, :],
                        scalar=g[:, m:m + 1], in1=rs_bc[:],
                        op0=ALU.mult, op1=ALU.mult)

            rmsnorm(semT_sb, M_SEM, TOK, gsem_sb, semn)
            rmsnorm(proT_sb, M_PRO, S, gpro_sb, pron)

        # ---------- QKV projections (fp8 DoubleRow) ----------
        es_qw = ExitStack()
        pqw = es_qw.enter_context(tc.tile_pool(name="pqw", bufs=1, side="right"))
        wq_sb = pqw.tile([P, M_SEM, M_SEM, P], FP8)
        wk_sb = pqw.tile([P, M_PRO, M_SEM, P], FP8)
        wv_sb = pqw.tile([P, M_PRO, DS], FP8)
        nc.sync.dma_start(wq_sb[:], wq[:])
        nc.sync.dma_start(wk_sb[:], wk[:])
        nc.sync.dma_start(wv_sb[:], wv[:])

        with tc.tile_pool(name="ps_mm", bufs=2, space="PSUM") as ps_mm:
            # Q: out q_sb[:, m, 0:1024]
            for m in range(M_SEM):
                ps = ps_mm.tile([P, 1024], F32, tag="mm")
                for n2 in range(2):
                    for kk in range(4):
                        nc.tensor.matmul(
                            ps[:, n2 * 512:(n2 + 1) * 512],
                            wq_sb[:, 2 * kk:2 * kk + 2, m, :],
                            semn[:, 2 * kk:2 * kk + 2, n2 * 512:(n2 + 1) * 512],
                            start=(kk == 0), stop=(kk == 3), perf_mode=DR)
                nc.vector.tensor_scalar(q_sb[:, m, :], ps[:], 1.0 / WS,
                                        bq_sb[:, m:m + 1], ALU.mult, ALU.add)
            # K: out k_sb[:, m, 0:2048]
            for m in range(M_SEM):
                for c2 in range(2):
                    ps = ps_mm.tile([P, 1024], F32, tag="mm")
                    for n2 in range(2):
                        col = (c2 * 2 + n2) * 512
                        for kk in range(2):
                            nc.tensor.matmul(
                                ps[:, n2 * 512:(n2 + 1) * 512],
                                wk_sb[:, 2 * kk:2 * kk + 2, m, :],
                                pron[:, 2 * kk:2 * kk + 2, col:col + 512],
                                start=(kk == 0), stop=(kk == 1), perf_mode=DR)
                    nc.scalar.activation(k_sb[:, m, c2 * 1024:(c2 + 1) * 1024],
                                         ps[:], AF.Identity, scale=1.0 / WS,
                                         bias=bk_sb[:, m:m + 1])
            # V (transposed, kpos-major): out v_sb[:, mt, 0:1024]
            for mt in range(NT_K):
                ps = ps_mm.tile([P, 1024], F32, tag="mm")
                for n2 in range(2):
                    for kk in range(2):
                        nc.tensor.matmul(
                            ps[:, n2 * 512:(n2 + 1) * 512],
                            pron[:, 2 * kk:2 * kk + 2, mt * P:(mt + 1) * P],
                            wv_sb[:, 2 * kk:2 * kk + 2, n2 * 512:(n2 + 1) * 512],
                            start=(kk == 0), stop=(kk == 1), perf_mode=DR)
                nc.vector.tensor_scalar(v_sb[:, mt, :], ps[:], 1.0 / WS, None,
                                        ALU.mult)
        es_norm.close()
        es_qw.close()

        # FFN weights arrive during attention
        es_fw = ExitStack()
        pfw = es_fw.enter_context(tc.tile_pool(name="pfw", bufs=1, side="right"))
        w1q_sb = pfw.tile([P, M_SEM, M_FF, P], FP8)
        w2q_sb = pfw.tile([P, M_FF, M_SEM, P], FP8)
        for c4 in range(4):
            nc.sync.dma_start(w1q_sb[:, 2 * c4:2 * c4 + 2], w1q[:, 2 * c4:2 * c4 + 2])
        for c4 in range(4):
            nc.sync.dma_start(w2q_sb[:, 8 * c4:8 * c4 + 8], w2q[:, 8 * c4:8 * c4 + 8])
        wo_sb = pfw.tile([P, M_SEM, M_SEM, P], FP8)
        nc.sync.dma_start(wo_sb[:], wo[:])

        # ---------- attention ----------
        with tc.tile_pool(name="pattn", bufs=1) as pattn, \
             tc.tile_pool(name="ps_att", bufs=1, space="PSUM") as ps_att:
            for n in range(2):
                for h in range(H):
                    pt = pattn.tile([P, NT_K, 512], FP8, tag="pt", bufs=3)
                    den = ps_att.tile([P, 512], F32, tag="den", bufs=2)
                    cps = ps_att.tile([P, 512], F32, tag="pv", bufs=2)
                    for kp in range(8):
                        sc = ps_att.tile([P, 1024], F32, tag="sc", bufs=2)
                        nc.tensor.matmul(
                            sc[:, 0:512],
                            k_sb[:, h, (2 * kp) * P:(2 * kp + 1) * P],
                            q_sb[:, h, n * 512:(n + 1) * 512],
                            start=True, stop=True)
                        nc.tensor.matmul(
                            sc[:, 512:1024],
                            k_sb[:, h, (2 * kp + 1) * P:(2 * kp + 2) * P],
                            q_sb[:, h, n * 512:(n + 1) * 512],
                            start=True, stop=True)
                        nc.scalar.activation(
                            pt[:, 2 * kp:2 * kp + 2, :].rearrange(
                                "p a b -> p (a b)"),
                            sc[:], AF.Exp, scale=QK_SCALE)
                        nc.tensor.matmul(den[:], ones2_f8[:],
                                         pt[:, 2 * kp:2 * kp + 2, :],
                                         start=(kp == 0), stop=(kp == 7),
                                         perf_mode=DR)
                        nc.tensor.matmul(cps[:],
                                         v_sb[:, 2 * kp:2 * kp + 2,
                                              h * P:(h + 1) * P],
                                         pt[:, 2 * kp:2 * kp + 2, :],
                                         start=(kp == 0), stop=(kp == 7),
                                         perf_mode=DR)
                    bc_sb = pattn.tile([P, 512], F32, tag="bcsb", bufs=2)
                    nc.vector.reciprocal_approx_fast(bc_sb[:], den[:])
                    nc.vector.tensor_tensor(ctx_n[n][:, h, :], cps[:], bc_sb[:],
                                            op=ALU.mult)
        es_att.close()

        # ---------- out-proj + ffn-norm + FFN (per half) ----------
        with tc.tile_pool(name="pff", bufs=1) as pff, \
             tc.tile_pool(name="ps_ff", bufs=2, space="PSUM") as ps_ff:
            xq_n = [pff.tile([P, M_SEM, 512], FP8, name=f"xq{n}")
                    for n in range(2)]
            h2_n = [pff.tile([P, M_FF, 512], FP8, name=f"h2{n}")
                    for n in range(2)]

            def oproj(n):
                ncol = slice(n * 512, (n + 1) * 512)
                t_o = pff.tile([P, M_SEM, 512], BF16, tag="t_o", bufs=2)
                for m in range(M_SEM):
                    ps = ps_ff.tile([P, 512], F32, tag="mmf")
                    for kk in range(4):
                        nc.tensor.matmul(ps[:], wo_sb[:, 2 * kk:2 * kk + 2, m, :],
                                         ctx_n[n][:, 2 * kk:2 * kk + 2, :],
                                         start=(kk == 0), stop=(kk == 3),
                                         perf_mode=DR)
                    nc.vector.tensor_scalar(t_o[:, m, :], ps[:],
                                            1.0 / (WS * CTX_S),
                                            obias_sb[:, m:m + 1],
                                            ALU.mult, ALU.add)
                semo = semT_sb[:, :, ncol]
                nc.vector.tensor_tensor(semo, t_o[:], semo, op=ALU.add)

            def xnorm(n):
                ncol = slice(n * 512, (n + 1) * 512)
                semo = semT_sb[:, :, ncol]
                sqf = pff.tile([P, M_SEM, 512], BF16, tag="sqf", bufs=1)
                nc.scalar.activation(sqf[:], semo, AF.Square)
                pst = ps_ff.tile([P, 512], F32, tag="nrmf")
                for m in range(M_SEM):
                    nc.tensor.matmul(pst[:], ones_full[:], sqf[:, m, :],
                                     start=(m == 0), stop=(m == M_SEM - 1))
                rsbc = pff.tile([P, 512], F32, tag="rsbc", bufs=1)
                nc.scalar.activation(rsbc[:], pst[:], AF.Ln, bias=eps_t[:],
                                     scale=1.0 / DS)
                nc.scalar.activation(rsbc[:], rsbc[:], AF.Exp, scale=-0.5)
                for m in range(M_SEM):
                    nc.vector.scalar_tensor_tensor(
                        out=xq_n[n][:, m, :], in0=semT_sb[:, m, ncol],
                        scalar=gff_sb[:, m:m + 1], in1=rsbc[:],
                        op0=ALU.mult, op1=ALU.mult)

            def ffn1(n):
                for m in range(M_FF):
                    ps = ps_ff.tile([P, 512], F32, tag="mmf")
                    for kk in range(4):
                        nc.tensor.matmul(ps[:], w1q_sb[:, 2 * kk:2 * kk + 2, m, :],
                                         xq_n[n][:, 2 * kk:2 * kk + 2, :],
                                         start=(kk == 0), stop=(kk == 3),
                                         perf_mode=DR)
                    sn = pff.tile([P, 512], BF16, tag="sn", bufs=3)
                    nc.scalar.activation(sn[:], ps[:], AF.Sin, bias=hpi_t[:],
                                         scale=acos_sb[:, m:m + 1])
                    nc.vector.scalar_tensor_tensor(
                        out=h2_n[n][:, m, :], in0=sn[:],
                        scalar=nrb2_sb[:, m:m + 1], in1=ps[:],
                        op0=ALU.mult, op1=ALU.add)

            def ffn2(n):
                ncol = slice(n * 512, (n + 1) * 512)
                yf = pff.tile([P, M_SEM, 512], BF16, tag="yf", bufs=1)
                for m in range(M_SEM):
                    ps = ps_ff.tile([P, 512], F32, tag="mmf")
                    for kk in range(16):
                        nc.tensor.matmul(ps[:], w2q_sb[:, 2 * kk:2 * kk + 2, m, :],
                                         h2_n[n][:, 2 * kk:2 * kk + 2, :],
                                         start=(kk == 0), stop=(kk == 15),
                                         perf_mode=DR)
                    nc.vector.tensor_scalar(yf[:, m, :], ps[:], dqs_sb[:],
                                            ybias_sb[:, m:m + 1],
                                            ALU.mult, ALU.add)
                yo = pff.tile([P, M_SEM, 512], F32, tag="yo", bufs=1)
                for m2 in range(4):
                    nc.vector.tensor_tensor(
                        yo[:, 2 * m2:2 * m2 + 2, :],
                        yf[:, 2 * m2:2 * m2 + 2, :],
                        semT_sb[:, 2 * m2:2 * m2 + 2, ncol], op=ALU.add)
                    nc.sync.dma_start(
                        outT_r[:, 2 * m2:2 * m2 + 2, ncol],
                        yo[:, 2 * m2:2 * m2 + 2, :])

            oproj(0)
            xnorm(0)
            oproj(1)
            ffn1(0)
            xnorm(1)
            ffn2(0)
            ffn1(1)
            ffn2(1)
        es_fw.close()

    nc.compile()
    return nc


_NC_CACHE = {}


def _get_nc(debug_outs=False):
    key = bool(debug_outs)
    if key not in _NC_CACHE:
        _NC_CACHE[key] = build_nc(debug_outs)
    return _NC_CACHE[key]


def make_in_maps(inputs):
    """Host-side shard + layout prep. inputs: dict of full np arrays."""
    import ml_dtypes
    f8 = ml_dtypes.float8_e4m3
    f32 = np.float32
    sem = np.asarray(inputs["sem"], f32)
    pro = np.asarray(inputs["pro"], f32)

    def cols(v, nm):
        return np.ascontiguousarray(np.asarray(v, f32).reshape(nm, P).T)

    def wlay(wT, nkt, nm):
        # [in=nkt*128, out=nm*128] -> [128p, nkt, nm, 128c]
        return np.ascontiguousarray(
            wT.reshape(nkt, P, nm, P).transpose(1, 0, 2, 3))

    WqT = np.asarray(inputs["Wq"], f32).T * WS
    WkT = np.asarray(inputs["Wk"], f32).T * WS
    WvT = np.asarray(inputs["Wv"], f32).T * WS
    WoT = np.asarray(inputs["Wo"], f32).T * WS

    W1 = np.asarray(inputs["W1"], f32)
    W2 = np.asarray(inputs["W2"], f32)
    mw1 = np.maximum(np.abs(W1).mean(), 1e-5)
    mw2 = np.maximum(np.abs(W2).mean(), 1e-5)
    w1s = np.clip(np.round(W1 / mw1), -1, 1)          # [DF, DS]
    w2s = np.clip(np.round(W2 / mw2), -1, 1)          # [DS, DF]

    alpha = np.asarray(inputs["alpha"], f32)
    beta = np.asarray(inputs["beta"], f32)
    acos_v = 2.0 * alpha * mw1
    rb2 = 1.0 / (2.0 * (beta + 1e-9) * mw1)           # positive snake const
    ybias_v = (mw1 * mw2) * (w2s @ rb2)               # [DS]
    obias_v = np.asarray(inputs["bo"], f32) + \
        np.asarray(inputs["Wo"], f32) @ np.asarray(inputs["bv"], f32)

    common = {
        "gsem": cols(inputs["g_sem"], M_SEM),
        "gpro": cols(inputs["g_pro"], M_PRO),
        "gff": cols(inputs["g_ff"], M_SEM),
        "bq": cols(inputs["bq"], M_SEM),
        "bk": cols(inputs["bk"], M_SEM),
        "obias": cols(obias_v, M_SEM),
        "ybias": cols(ybias_v, M_SEM),
        "acos": cols(acos_v, M_FF),
        "nrb2": cols(-rb2, M_FF),
        "dqs": np.full((P, 1), mw1 * mw2, f32),
        "wq": wlay(WqT, M_SEM, M_SEM).astype(f8),
        "wk": wlay(WkT, M_PRO, M_SEM).astype(f8),
        "wv": np.ascontiguousarray(
            WvT.reshape(M_PRO, P, DS).transpose(1, 0, 2)).astype(f8),
        "wo": wlay(WoT, M_SEM, M_SEM).astype(f8),
        "w1q": wlay(np.ascontiguousarray(w1s.T), M_SEM, M_FF).astype(f8),
        "w2q": wlay(np.ascontiguousarray(w2s.T), M_FF, M_SEM).astype(f8),
    }

    in_maps = []
    for c in range(N_CORES):
        b, half = c // 2, c % 2
        m = dict(common)
        m["semT"] = np.ascontiguousarray(sem[b, half * TOK:(half + 1) * TOK, :].T)
        m["proT"] = np.ascontiguousarray(pro[b].T).astype(ml_dtypes.bfloat16)
        in_maps.append(m)
    return in_maps


def assemble_out(results):
    out = np.empty((B, S, DS), np.float32)
    for c in range(N_CORES):
        b, half = c // 2, c % 2
        out[b, half * TOK:(half + 1) * TOK, :] = results[c]["outT"].T
    return out


def kernel(**inputs):
    nc = _get_nc()
    in_maps = make_in_maps(inputs)
    res = run_bass_kernel_spmd(nc, in_maps, core_ids=list(range(N_CORES)))
    return assemble_out(res.results)
